# revision 26
# baseline (speedup 1.0000x reference)
"""Trainium2 Bass kernel for nn_AdaptiveDepthRWKV (8 NeuronCores, SPMD).

Sharding: token-parallel backbone (core r owns timesteps [128r,128(r+1)) of both
batches; activations feature-major [d, tok] in SBUF), weights replicated and
streamed bf16 from HBM with per-partition-contiguous layouts. The RWKV
decay-cumsum runs as a native DVE tensor_tensor_scan recurrence
z_t = dec*z_{t-1} + b_t*kv_t with a tiny carry all-gather across cores.
GroupNorm uses a linearity trick: the Wo_tm matmul runs on un-normalized data,
then an istd*P + cvec rank-1 fixup lands after the stats all-reduce. Heads are
vocab-sharded (4000 cols/core) after all-gathering the LN'd hidden state. Loss
terms are assembled on host in f64 from the full logits.
"""
import sys

sys.path.insert(0, "/opt/trn_rl_repo")

import numpy as np
import ml_dtypes

import concourse.bass as bass
import concourse.mybir as mybir
import concourse.tile as tile
from concourse import bacc
from concourse.bass_utils import run_bass_kernel_spmd

F32 = mybir.dt.float32
BF16 = mybir.dt.bfloat16
AF = mybir.ActivationFunctionType
OP = mybir.AluOpType

V, D, DFF, L, B, T = 32000, 1024, 4096, 12, 2, 1024
R = 8                 # cores
TL = T // R           # 128 timesteps per core
NT = B * TL           # 256 local tokens, free order: [b0 t0..127, b1 t0..127]
VS = V // R           # 4000 vocab cols per core
DC = D // 128         # 8 hidden chunks
FC = DFF // 128       # 32 ffn chunks
NV = 8                # vocab chunks per core
VC = VS // NV         # 500
EXITS = (4, 8)
WEIGHTS = (0.3, 0.5, 1.0)

_LN_COLS = {}


def _lncol(name):
    _LN_COLS[name] = len(_LN_COLS)


_lncol("in_w"); _lncol("in_b")
for _i in range(L):
    for _n in ("ln1_w", "ln1_b", "ln2_w", "ln2_b"):
        _lncol(f"{_n}_{_i}")
_lncol("out_w"); _lncol("out_b")
for _j in range(2):
    _lncol(f"ex_w_{_j}"); _lncol(f"ex_b_{_j}")
NLN = len(_LN_COLS)


def _build(n_layers=L, do_heads=True, dbg_x=False, stage=99):
    nc = bacc.Bacc("TRN2", target_bir_lowering=False, debug=False, num_devices=R)

    # ---------------- inputs ----------------
    xembT = nc.dram_tensor("xembT", [D, NT], F32, kind="ExternalInput")
    lnp_d = nc.dram_tensor("lnp", [128, DC, NLN], F32, kind="ExternalInput")
    decayT_d = nc.dram_tensor("decayT", [128, DC, L], F32, kind="ExternalInput")
    gnaux_d = nc.dram_tensor("gnaux", [128, DC, 2 * L], F32, kind="ExternalInput")
    iota_d = nc.dram_tensor("iota", [1, TL], F32, kind="ExternalInput")
    rmask_d = nc.dram_tensor("rmask", [1, R], F32, kind="ExternalInput")
    # weights repacked [out_hi, in_lo(128), in_hi, out_lo] (partition-contiguous)
    w_qkv = [[nc.dram_tensor(f"w{n}{i}", [DC, 128, DC, 128], BF16,
                             kind="ExternalInput")
              for n in ("r", "k", "v", "o")] for i in range(n_layers)]
    w_12 = [[nc.dram_tensor(f"wf{n}{i}", [FC, 128, DC, 128], BF16,
                            kind="ExternalInput")
             for n in (1, 2)] for i in range(n_layers)]
    w_cm = [nc.dram_tensor(f"wcm{i}", [DC, 128, FC, 128], BF16,
                           kind="ExternalInput") for i in range(n_layers)]
    headT = [nc.dram_tensor(f"headT{j}", [NV, 128, DC, VC], BF16,
                            kind="ExternalInput")
             for j in range(3 if do_heads else 0)]

    # ---------------- outputs ----------------
    out_logits = [nc.dram_tensor(n, [B * T, VS], F32, kind="ExternalOutput")
                  for n in ("exit0", "exit1", "logits")]
    hemean_o = nc.dram_tensor("hemean", [128, 4, DC], F32, kind="ExternalOutput")
    xdbg_o = (nc.dram_tensor("xdbg", [128, DC, NT], F32, kind="ExternalOutput")
              if dbg_x else None)

    # ---------------- collective buffers ----------------
    cc_carry_in = [nc.dram_tensor(f"cyi{i}", [128, 2 * DC], F32) for i in range(L)]
    cc_carry_out = [nc.dram_tensor(f"cyo{i}", [128 * R, 2 * DC], F32,
                                   addr_space="Shared") for i in range(L)]
    cc_gn_in = [nc.dram_tensor(f"gni{i}", [1, 4], F32) for i in range(L)]
    cc_gn_out = [nc.dram_tensor(f"gno{i}", [1, 4], F32, addr_space="Shared")
                 for i in range(L)]
    heb = [nc.dram_tensor(f"heb{j}", [128, DC * NT], BF16) for j in range(3)]
    heg = [nc.dram_tensor(f"heg{j}", [128 * R, DC * NT], BF16,
                          addr_space="Shared") for j in range(3)]
    RG = [list(range(R))]

    with tile.TileContext(nc) as tc, \
         tc.tile_pool(name="persist", bufs=1) as pers, \
         tc.tile_pool(name="act1", bufs=1) as act1, \
         tc.tile_pool(name="act2", bufs=2) as act2, \
         tc.tile_pool(name="scratch", bufs=2) as scr, \
         tc.tile_pool(name="small", bufs=2) as smp, \
         tc.tile_pool(name="wdd", bufs=2) as wddp, \
         tc.tile_pool(name="wcmp", bufs=2) as wcmp, \
         tc.tile_pool(name="whead", bufs=2) as wheadp, \
         tc.tile_pool(name="hegp", bufs=1) as hegp, \
         tc.tile_pool(name="ps1", bufs=1, space="PSUM") as pss, \
         tc.tile_pool(name="psa", bufs=2, space="PSUM") as psa, \
         tc.tile_pool(name="psb", bufs=2, space="PSUM") as psb, \
         tc.tile_pool(name="psd", bufs=2, space="PSUM") as psd:

        # ---- persistent setup ----
        x = pers.tile([128, DC, NT], F32)          # residual master, feature-major
        lnp = pers.tile([128, DC, NLN], F32)
        decayT = pers.tile([128, DC, L], F32)
        gnaux = pers.tile([128, DC, 2 * L], F32)
        iota_b = pers.tile([128, TL], F32)
        rmask_b = pers.tile([128, R], F32)
        ones_col = pers.tile([128, 1], F32)
        nc.sync.dma_start(lnp[:], lnp_d[:])
        nc.sync.dma_start(decayT[:], decayT_d[:])
        nc.sync.dma_start(gnaux[:], gnaux_d[:])
        iota_r = pers.tile([1, TL], F32)
        rmask_r = pers.tile([1, R], F32)
        nc.sync.dma_start(iota_r[:], iota_d[:])
        nc.sync.dma_start(rmask_r[:], rmask_d[:])
        nc.gpsimd.partition_broadcast(iota_b[:], iota_r[:])
        nc.gpsimd.partition_broadcast(rmask_b[:], rmask_r[:])
        nc.vector.memset(ones_col[:], 1.0)

        def lncol(name):
            return lnp[:, :, _LN_COLS[name]]

        # ---- feature-major layernorm ----
        def layer_norm(src, w_cols, b_cols, out_dtype, accum_cols=None,
                       out_tile=None):
            """src: [128, DC, NT] f32. Returns h [128, DC, NT] out_dtype."""
            sum_ps = pss.tile([1, NT], F32, tag="lnsum")
            sq_ps = pss.tile([1, NT], F32, tag="lnsq")
            for c in range(DC):
                sq = scr.tile([128, NT], F32, tag="lnsqt")
                nc.scalar.activation(sq[:], src[:, c, :], AF.Square)
                nc.tensor.matmul(sum_ps[:], ones_col[:], src[:, c, :],
                                 start=(c == 0), stop=(c == DC - 1))
                nc.tensor.matmul(sq_ps[:], ones_col[:], sq[:],
                                 start=(c == 0), stop=(c == DC - 1))
            mrow = smp.tile([1, NT], F32, tag="lnm")
            var = smp.tile([1, NT], F32, tag="lnv")
            msq = smp.tile([1, NT], F32, tag="lnmsq")
            nc.vector.tensor_scalar_mul(mrow[:], sum_ps[:], 1.0 / D)
            nc.vector.tensor_scalar_mul(var[:], sq_ps[:], 1.0 / D)
            nc.vector.tensor_tensor(msq[:], mrow[:], mrow[:], OP.mult)
            nc.vector.tensor_tensor(var[:], var[:], msq[:], OP.subtract)
            nc.vector.tensor_scalar_add(var[:], var[:], 1e-5)
            nc.scalar.sqrt(var[:], var[:])
            nc.vector.reciprocal(var[:], var[:])          # istd row
            nc.vector.tensor_tensor(mrow[:], mrow[:], var[:], OP.mult)  # m*istd
            ib = smp.tile([128, NT], F32, tag="lnib")
            mib = smp.tile([128, NT], F32, tag="lnmib")
            nc.gpsimd.partition_broadcast(ib[:], var[:])
            nc.gpsimd.partition_broadcast(mib[:], mrow[:])
            if out_tile is None:
                h = act2.tile([128, DC, NT], out_dtype, tag="h")
            else:
                h = out_tile
            for c in range(DC):
                t1 = scr.tile([128, NT], F32, tag="lnt1")
                nc.vector.tensor_tensor(t1[:], src[:, c, :], ib[:], OP.mult)
                nc.vector.tensor_tensor(t1[:], t1[:], mib[:], OP.subtract)
                if accum_cols is None:
                    nc.vector.tensor_scalar(h[:, c, :], t1[:],
                                            w_cols[:, c:c + 1], b_cols[:, c:c + 1],
                                            OP.mult, OP.add)
                else:
                    for b in range(B):
                        sl = slice(b * TL, (b + 1) * TL)
                        nc.vector.tensor_scalar(
                            h[:, c, sl], t1[:, sl],
                            w_cols[:, c:c + 1], b_cols[:, c:c + 1],
                            OP.mult, OP.add,
                            accum_out=accum_cols[b][:, c:c + 1])
            return h

        # ---- input embedding LN -> x ----
        xemb = act1.tile([128, DC, NT], F32, tag="kv")   # reuse kv slot
        nc.sync.dma_start(xemb[:], xembT.rearrange("(o p) f -> p o f", p=128))
        layer_norm(xemb, lncol("in_w"), lncol("in_b"), F32, out_tile=x)

        def dump(t):
            if xdbg_o is not None:
                nc.sync.dma_start(xdbg_o[:], t[:])

        # =================================================================
        for i in range(n_layers):
            # ---------------- TimeMix ----------------
            h = layer_norm(x, lncol(f"ln1_w_{i}"), lncol(f"ln1_b_{i}"), BF16)
            if stage == 0:
                hh = act1.tile([128, DC, NT], F32, tag="kv")
                for c in range(DC):
                    nc.vector.tensor_copy(hh[:, c, :], h[:, c, :])
                dump(hh)
                break

            # decay tables; w = min(exp(-t*lndec), 1e10), b = min(scale*1e10, 1)
            dec = smp.tile([128, DC], F32, tag="dec")
            lndec = smp.tile([128, DC], F32, tag="lndec")
            nc.scalar.activation(dec[:], decayT[:, :, i], AF.Sigmoid)
            nc.vector.tensor_scalar_max(dec[:], dec[:], 1e-7)
            nc.scalar.activation(lndec[:], dec[:], AF.Ln)
            scale_t = act1.tile([128, DC, TL], F32, tag="scale")
            for c in range(DC):
                sc0 = scr.tile([128, TL], F32, tag="sc0")
                nc.vector.tensor_scalar_mul(sc0[:], iota_b[:], lndec[:, c:c + 1])
                nc.scalar.activation(scale_t[:, c, :], sc0[:], AF.Exp)

            # k,v projections + kv product + carry sums
            kv = act1.tile([128, DC, NT], F32, tag="kv")
            r = act1.tile([128, DC, NT], F32, tag="r")
            ysum = smp.tile([128, 2 * DC], F32, tag="ysum")
            wkq = wvq = wrq = None
            for c in range(DC):
                if c % 2 == 0:
                    wkq = wddp.tile([128, 2, DC, 128], BF16, tag="wk", name="wk")
                    wvq = wddp.tile([128, 2, DC, 128], BF16, tag="wv", name="wv")
                    nc.sync.dma_start(wkq[:], w_qkv[i][1][c // 2 * 2:c // 2 * 2 + 2]
                                      .rearrange("q p o f -> p q o f"))
                    nc.sync.dma_start(wvq[:], w_qkv[i][2][c // 2 * 2:c // 2 * 2 + 2]
                                      .rearrange("q p o f -> p q o f"))
                ci = c % 2
                kps = psa.tile([128, NT], F32, tag="pa")
                vps = psb.tile([128, NT], F32, tag="pb")
                for d in range(DC):
                    nc.tensor.matmul(kps[:], wkq[:, ci, d, :], h[:, d, :],
                                     start=(d == 0), stop=(d == DC - 1))
                for d in range(DC):
                    nc.tensor.matmul(vps[:], wvq[:, ci, d, :], h[:, d, :],
                                     start=(d == 0), stop=(d == DC - 1))
                ksb = scr.tile([128, NT], F32, tag="ksb")
                nc.scalar.copy(ksb[:], kps[:])
                nc.vector.tensor_tensor(kv[:, c, :], ksb[:], vps[:], OP.mult)
                wtab = scr.tile([128, TL], F32, tag="wtab")
                nc.vector.tensor_scalar(wtab[:], iota_b[:], lndec[:, c:c + 1],
                                        -1.0, OP.mult, OP.mult)
                nc.scalar.activation(wtab[:], wtab[:], AF.Exp)
                nc.vector.tensor_scalar_min(wtab[:], wtab[:], 1e10)
                for b in range(B):
                    sc = scr.tile([128, TL], F32, tag="ysc")
                    nc.vector.tensor_tensor(sc[:], kv[:, c, b * TL:(b + 1) * TL],
                                            wtab[:], OP.mult)
                    nc.vector.tensor_reduce(
                        ysum[:, c * 2 + b:c * 2 + b + 1], sc[:],
                        axis=mybir.AxisListType.X, op=OP.add)
            if stage == 1:
                dump(kv)
                break
            nc.sync.dma_start(cc_carry_in[i][:], ysum[:])
            nc.gpsimd.collective_compute(
                "AllGather", OP.bypass, replica_groups=RG,
                ins=[cc_carry_in[i].ap().opt()], outs=[cc_carry_out[i].ap().opt()])

            for c in range(DC):
                if c % 2 == 0:
                    wrq = wddp.tile([128, 2, DC, 128], BF16, tag="wr", name="wr")
                    nc.sync.dma_start(wrq[:], w_qkv[i][0][c // 2 * 2:c // 2 * 2 + 2]
                                      .rearrange("q p o f -> p q o f"))
                rps = psa.tile([128, NT], F32, tag="pa")
                for d in range(DC):
                    nc.tensor.matmul(rps[:], wrq[:, c % 2, d, :], h[:, d, :],
                                     start=(d == 0), stop=(d == DC - 1))
                nc.scalar.activation(r[:, c, :], rps[:], AF.Sigmoid)

            # local scan (z written in place over kv)
            z = kv
            for c in range(DC):
                btab = scr.tile([128, TL], F32, tag="btab")
                nc.vector.tensor_scalar(btab[:], scale_t[:, c, :], 1e10, 1.0,
                                        OP.mult, OP.min)
                for b in range(B):
                    sl = slice(b * TL, (b + 1) * TL)
                    bkv = scr.tile([128, TL], F32, tag="bkv")
                    nc.vector.tensor_tensor(bkv[:], kv[:, c, sl], btab[:],
                                            OP.mult)
                    nc.vector.tensor_tensor_scan(
                        z[:, c, sl], dec[:, c:c + 1].to_broadcast((128, TL)),
                        bkv[:], 0.0, OP.mult, OP.add)
            if stage == 2:
                dump(z)
                break

            # carry prefix
            carry_all = smp.tile([128, R, 2 * DC], F32, tag="carry_all")
            nc.sync.dma_start(carry_all[:],
                              cc_carry_out[i].rearrange("(r p) f -> p r f", p=128))
            cmask = smp.tile([128, R, 2 * DC], F32, tag="cmask")
            nc.vector.tensor_tensor(
                cmask[:], carry_all[:],
                rmask_b[:, :, None].to_broadcast((128, R, 2 * DC)), OP.mult)
            cprev = smp.tile([128, 2 * DC], F32, tag="cprev")
            nc.vector.tensor_reduce(cprev[:],
                                    cmask[:].rearrange("p r f -> p f r"),
                                    axis=mybir.AxisListType.X, op=OP.add)

            # out = r * (scale*cprev + z); stats + bf16 cast (all DVE)
            out_bf = act1.tile([128, DC, NT], BF16, tag="out_bf")
            ssum = smp.tile([128, 2 * DC], F32, tag="ssum")
            ssq = smp.tile([128, 2 * DC], F32, tag="ssq")
            nc.vector.memset(ssum[:], 0.0)
            nc.vector.memset(ssq[:], 0.0)
            for c in range(DC):
                of = scr.tile([128, NT], F32, tag="outf")
                for b in range(B):
                    sl = slice(b * TL, (b + 1) * TL)
                    j = c * 2 + b
                    nc.vector.scalar_tensor_tensor(
                        of[:, sl], scale_t[:, c, :], cprev[:, j:j + 1],
                        z[:, c, sl], OP.mult, OP.add)
                    nc.vector.tensor_tensor(of[:, sl], r[:, c, sl], of[:, sl],
                                            OP.mult)
                    nc.vector.tensor_scalar(out_bf[:, c, sl], of[:, sl],
                                            1.0, 0.0, OP.mult, OP.add,
                                            accum_out=ssum[:, j:j + 1])
                    sq2 = scr.tile([128, TL], F32, tag="gnsq")
                    nc.vector.scalar_tensor_tensor(
                        sq2[:], of[:, sl], 1.0, of[:, sl], OP.mult, OP.mult,
                        accum_out=ssq[:, j:j + 1])
            if stage == 3:
                oo = act1.tile([128, DC, NT], F32, tag="kv")
                for c in range(DC):
                    nc.vector.tensor_copy(oo[:, c, :], out_bf[:, c, :])
                dump(oo)
                break

            # GN stats partition-reduce + AllReduce
            sgn_ps = pss.tile([1, NT], F32, tag="lnsum")
            sqn_ps = pss.tile([1, NT], F32, tag="lnsq")
            nc.tensor.matmul(sgn_ps[:, :2 * DC], ones_col[:], ssum[:])
            nc.tensor.matmul(sqn_ps[:, :2 * DC], ones_col[:], ssq[:])
            st_row = smp.tile([1, 4], F32, tag="gnrow")
            nc.vector.tensor_reduce(
                st_row[:, 0:2],
                sgn_ps[:, :2 * DC].rearrange("p (c b) -> p b c", b=B),
                axis=mybir.AxisListType.X, op=OP.add)
            nc.vector.tensor_reduce(
                st_row[:, 2:4],
                sqn_ps[:, :2 * DC].rearrange("p (c b) -> p b c", b=B),
                axis=mybir.AxisListType.X, op=OP.add)
            nc.sync.dma_start(cc_gn_in[i][:], st_row[:])
            nc.gpsimd.collective_compute(
                "AllReduce", OP.add, replica_groups=RG,
                ins=[cc_gn_in[i].ap().opt()], outs=[cc_gn_out[i].ap().opt()])

            # finish GN stats: istd / -m*istd per batch
            g_st = smp.tile([1, 4], F32, tag="gst")
            nc.sync.dma_start(g_st[:], cc_gn_out[i][:])
            inv = 1.0 / (T * D)
            mrow = smp.tile([1, 2], F32, tag="gm")
            vrow = smp.tile([1, 2], F32, tag="gv")
            t2 = smp.tile([1, 2], F32, tag="gt2")
            nc.vector.tensor_scalar_mul(mrow[:], g_st[:, 0:2], inv)
            nc.vector.tensor_scalar_mul(vrow[:], g_st[:, 2:4], inv)
            nc.vector.tensor_tensor(t2[:], mrow[:], mrow[:], OP.mult)
            nc.vector.tensor_tensor(vrow[:], vrow[:], t2[:], OP.subtract)
            nc.vector.tensor_scalar_add(vrow[:], vrow[:], 1e-5)
            nc.scalar.sqrt(vrow[:], vrow[:])
            nc.vector.reciprocal(vrow[:], vrow[:])           # istd [1,2]
            nc.vector.tensor_tensor(mrow[:], mrow[:], vrow[:], OP.mult)
            nc.vector.tensor_scalar_mul(mrow[:], mrow[:], -1.0)  # -m*istd
            row4 = smp.tile([1, 4], F32, tag="grow4")
            nc.vector.tensor_copy(row4[:, 0:2], vrow[:])
            nc.vector.tensor_copy(row4[:, 2:4], mrow[:])
            bc4 = smp.tile([128, 4], F32, tag="gbc4")
            nc.gpsimd.partition_broadcast(bc4[:], row4[:])
            cvec = smp.tile([128, DC, B], F32, tag="cvec")
            for b in range(B):
                nc.vector.tensor_scalar_mul(cvec[:, :, b], gnaux[:, :, 2 * i],
                                            bc4[:, 2 + b:3 + b])
                nc.vector.tensor_tensor(cvec[:, :, b], cvec[:, :, b],
                                        gnaux[:, :, 2 * i + 1], OP.add)

            # P = out_bf @ WoT (gn_w folded); fixup; residual add
            woq = None
            for d in range(DC):
                if d % 2 == 0:
                    woq = wddp.tile([128, 2, DC, 128], BF16, tag="wo", name="wo")
                    nc.sync.dma_start(woq[:], w_qkv[i][3][d // 2 * 2:d // 2 * 2 + 2]
                                      .rearrange("q p o f -> p q o f"))
                P = psd.tile([128, NT], F32, tag="pd")
                for c in range(DC):
                    nc.tensor.matmul(P[:], woq[:, d % 2, c, :], out_bf[:, c, :],
                                     start=(c == 0), stop=(c == DC - 1))
                for b in range(B):
                    sl = slice(b * TL, (b + 1) * TL)
                    nc.vector.tensor_scalar(P[:, sl], P[:, sl],
                                            bc4[:, b:b + 1], cvec[:, d, b:b + 1],
                                            OP.mult, OP.add)
                nc.vector.tensor_tensor(x[:, d, :], x[:, d, :], P[:], OP.add)
            if stage == 4:
                dump(x)
                break

            # ---------------- ChannelMix ----------------
            h2 = layer_norm(x, lncol(f"ln2_w_{i}"), lncol(f"ln2_b_{i}"), BF16)
            for gh in range(2):                       # ffn halves
                g = act1.tile([128, FC // 2, NT], BF16, tag="g", name="g")
                for fg in range(4):                   # groups of 4 f-tiles
                    ft0 = gh * (FC // 2) + fg * 4
                    w1c = wddp.tile([128, 4, DC, 128], BF16, tag="w1", name="w1")
                    w2c = wddp.tile([128, 4, DC, 128], BF16, tag="w2", name="w2")
                    nc.sync.dma_start(w1c[:], w_12[i][0][ft0:ft0 + 4]
                                      .rearrange("q p o f -> p q o f"))
                    nc.sync.dma_start(w2c[:], w_12[i][1][ft0:ft0 + 4]
                                      .rearrange("q p o f -> p q o f"))
                    for fi in range(4):
                        aps_ = psa.tile([128, NT], F32, tag="pa")
                        bps_ = psb.tile([128, NT], F32, tag="pb")
                        for d in range(DC):
                            nc.tensor.matmul(aps_[:], w1c[:, fi, d, :],
                                             h2[:, d, :],
                                             start=(d == 0), stop=(d == DC - 1))
                        for d in range(DC):
                            nc.tensor.matmul(bps_[:], w2c[:, fi, d, :],
                                             h2[:, d, :],
                                             start=(d == 0), stop=(d == DC - 1))
                        sa = scr.tile([128, NT], F32, tag="silu")
                        nc.scalar.activation(sa[:], aps_[:], AF.Silu)
                        nc.vector.tensor_tensor(g[:, fg * 4 + fi, :], sa[:],
                                                bps_[:], OP.mult)
                for d in range(DC):
                    wcq = wcmp.tile([128, FC // 2, 128], BF16, tag="wcq")
                    nc.sync.dma_start(
                        wcq[:], w_cm[i][d, :, gh * (FC // 2):(gh + 1) * (FC // 2), :])
                    d2 = psd.tile([128, NT], F32, tag="pd")
                    for fh in range(FC // 2):
                        nc.tensor.matmul(d2[:], wcq[:, fh, :], g[:, fh, :],
                                         start=(fh == 0), stop=(fh == FC // 2 - 1))
                    nc.vector.tensor_tensor(x[:, d, :], x[:, d, :], d2[:], OP.add)

            # ---------------- early exits ----------------
            if do_heads and (i + 1) in EXITS:
                j = EXITS.index(i + 1)
                _head(nc, layer_norm, lncol, smp, act2, hegp, wheadp, psa, scr,
                      x, j, heb[j], heg[j], headT[j], out_logits[j], RG,
                      hemean_o)

        # ---------------- final head ----------------
        if do_heads:
            _head(nc, layer_norm, lncol, smp, act2, hegp, wheadp, psa, scr,
                  x, 2, heb[2], heg[2], headT[2], out_logits[2], RG, hemean_o)
        if xdbg_o is not None and stage > 90:
            nc.sync.dma_start(xdbg_o[:], x[:])

    nc.compile()
    return nc


def _head(nc, layer_norm, lncol, smp, act2, hegp, wheadp, psa, scr,
          x, j, heb_d, heg_d, headT_d, out_d, RG, hemean_o):
    """LN -> all-gather he -> vocab-sharded logits matmul -> DMA out."""
    if j < 2:
        wc, bc = lncol(f"ex_w_{j}"), lncol(f"ex_b_{j}")
        hm = [smp.tile([128, DC], F32, tag=f"hm{j}{b}", name=f"hm{j}{b}")
              for b in range(B)]
        for b in range(B):
            nc.vector.memset(hm[b][:], 0.0)
        he = layer_norm(x, wc, bc, BF16, accum_cols=hm)
        for b in range(B):
            nc.sync.dma_start(hemean_o[:, j * 2 + b, :], hm[b][:])
    else:
        he = layer_norm(x, lncol("out_w"), lncol("out_b"), BF16)
    nc.sync.dma_start(heb_d.rearrange("p (o f) -> p o f", o=DC), he[:])
    nc.gpsimd.collective_compute(
        "AllGather", OP.bypass, replica_groups=RG,
        ins=[heb_d.ap().opt()], outs=[heg_d.ap().opt()])
    heg_sb = hegp.tile([128, R, DC, NT], BF16, tag="heg")
    nc.sync.dma_start(heg_sb[:],
                      heg_d.rearrange("(r p) (o f) -> p r o f", p=128, o=DC))
    for vc in range(NV):
        wh = wheadp.tile([128, DC, VC], BF16, tag="wh")
        nc.sync.dma_start(wh[:], headT_d[vc])
        for mt in range(2 * R):
            rb, jh = mt // 2, mt % 2
            lps = psa.tile([128, VC], F32, tag="pa")
            for d in range(DC):
                nc.tensor.matmul(
                    lps[:], heg_sb[:, rb, d, jh * 128:(jh + 1) * 128],
                    wh[:, d, :], start=(d == 0), stop=(d == DC - 1))
            ls = scr.tile([128, VC], F32, tag="lsb")
            nc.scalar.copy(ls[:], lps[:])
            nc.sync.dma_start(
                out_d[mt * 128:(mt + 1) * 128, vc * VC:(vc + 1) * VC], ls[:])


# =====================================================================
# host side
# =====================================================================
_CACHE = {}


def _get_nc():
    if "nc" not in _CACHE:
        _CACHE["nc"] = _build()
    return _CACHE["nc"]


def _bf(a):
    return np.ascontiguousarray(a.astype(ml_dtypes.bfloat16))


def _f32(a):
    return np.ascontiguousarray(np.asarray(a, dtype=np.float32))


def _featmaj(vec):
    """[D] -> [128, DC] feature-major column block."""
    return np.ascontiguousarray(vec.reshape(DC, 128).T)


def _repack(Wout_in):
    """W [n_out, n_in] -> [out_hi, in_lo(128), in_hi, out_lo(128)] bf16."""
    no, ni = Wout_in.shape
    a = Wout_in.reshape(no // 128, 128, ni // 128, 128)
    return _bf(a.transpose(0, 3, 2, 1))


def _repack_cm(Wdf):
    """Wo_cm [D, DFF] -> [d_hi, f_lo(128), f_hi, d_lo(128)] bf16."""
    a = Wdf.reshape(DC, 128, FC, 128)
    return _bf(a.transpose(0, 3, 2, 1))


def _repack_head(Wvd):
    """W [VS, D] -> [NV, d_lo(128), DC, VC] bf16."""
    a = Wvd.reshape(NV, VC, DC, 128)
    return _bf(a.transpose(0, 3, 2, 1))


def kernel(idx, targets, embed, ln_in_w, ln_in_b, Wr, Wk, Wv, Wo_tm, decay,
           gn_w, gn_b, ln1_w, ln1_b, ln2_w, ln2_b, W1, W2, Wo_cm,
           ln_out_w, ln_out_b, exit_ln_w, exit_ln_b, exit_head,
           gate_w1, gate_b1, gate_w2, gate_b2):
    idx = np.asarray(idx)
    targets = np.asarray(targets)
    embed = _f32(embed)
    nc = _get_nc()

    # ---- shared (rank-independent) prep ----
    lnp = np.zeros((128, DC, NLN), np.float32)

    def setln(name, vec):
        lnp[:, :, _LN_COLS[name]] = _featmaj(_f32(vec))

    setln("in_w", ln_in_w); setln("in_b", ln_in_b)
    for i in range(L):
        setln(f"ln1_w_{i}", ln1_w[i]); setln(f"ln1_b_{i}", ln1_b[i])
        setln(f"ln2_w_{i}", ln2_w[i]); setln(f"ln2_b_{i}", ln2_b[i])
    setln("out_w", ln_out_w); setln("out_b", ln_out_b)
    for j in range(2):
        setln(f"ex_w_{j}", exit_ln_w[j]); setln(f"ex_b_{j}", exit_ln_b[j])

    decay = _f32(decay)
    decayT = np.ascontiguousarray(
        np.stack([_featmaj(decay[i]) for i in range(L)], axis=-1))

    gnaux = np.zeros((128, DC, 2 * L), np.float32)
    Wo_tm = _f32(Wo_tm); gn_w = _f32(gn_w); gn_b = _f32(gn_b)
    for i in range(L):
        gnaux[:, :, 2 * i] = _featmaj(Wo_tm[i] @ gn_w[i])
        gnaux[:, :, 2 * i + 1] = _featmaj(Wo_tm[i] @ gn_b[i])

    shared = {"lnp": lnp, "decayT": decayT, "gnaux": gnaux}
    for i in range(L):
        shared[f"wr{i}"] = _repack(_f32(Wr[i]))
        shared[f"wk{i}"] = _repack(_f32(Wk[i]))
        shared[f"wv{i}"] = _repack(_f32(Wv[i]))
        shared[f"wo{i}"] = _repack(Wo_tm[i] * gn_w[i][None, :])
        shared[f"wf1{i}"] = _repack(_f32(W1[i]))
        shared[f"wf2{i}"] = _repack(_f32(W2[i]))
        shared[f"wcm{i}"] = _repack_cm(_f32(Wo_cm[i]))

    x_emb = embed[np.asarray(idx, dtype=np.int64)]        # [B, T, D]
    exit_head = _f32(exit_head)

    in_maps = []
    for r in range(R):
        m = dict(shared)
        sl = x_emb[:, r * TL:(r + 1) * TL, :]              # [B, TL, D]
        m["xembT"] = np.ascontiguousarray(
            sl.transpose(2, 0, 1).reshape(D, NT).astype(np.float32))
        m["iota"] = (np.arange(TL, dtype=np.float32) + r * TL).reshape(1, TL)
        m["rmask"] = (np.arange(R) < r).astype(np.float32).reshape(1, R)
        vs = slice(r * VS, (r + 1) * VS)
        m["headT0"] = _repack_head(exit_head[0, vs, :])
        m["headT1"] = _repack_head(exit_head[1, vs, :])
        m["headT2"] = _repack_head(embed[vs, :])
        in_maps.append(m)

    _CACHE["in_maps"] = in_maps
    res = run_bass_kernel_spmd(nc, in_maps, core_ids=list(range(R)))

    # ---- unshard ----
    def assemble(name):
        shards = []
        for r in range(R):
            a = res.results[r][name]                       # [2048, VS]
            a = a.reshape(R, B, TL, VS).transpose(1, 0, 2, 3).reshape(B, T, VS)
            shards.append(a)
        return np.concatenate(shards, axis=-1)             # [B, T, V]

    final_logits = assemble("logits")
    e_logits = [assemble("exit0"), assemble("exit1")]

    hm = np.zeros((2, B, D), np.float64)
    for r in range(R):
        a = res.results[r]["hemean"]                        # [128, 4, DC]
        for j in range(2):
            for b in range(B):
                hm[j, b] += a[:, j * 2 + b, :].T.reshape(D).astype(np.float64)
    hm /= T

    # ---- loss in f64 ----
    tgt = np.asarray(targets, dtype=np.int64)

    def ce_and_stats(lg):
        lg = lg.astype(np.float64)
        mx = lg.max(-1, keepdims=True)
        ex = np.exp(lg - mx)
        Z = ex.sum(-1, keepdims=True)
        lse = (mx + np.log(Z))[..., 0]                     # [B,T]
        tl_ = np.take_along_axis(lg, tgt[..., None], -1)[..., 0]
        ce = float((lse - tl_).mean())
        pred = lg.argmax(-1)
        p = ex / Z
        ent = lse - (p * lg).sum(-1)                       # [B,T]
        return ce, pred, ent

    ce_f, pred_f, _ = ce_and_stats(final_logits)
    loss = WEIGHTS[-1] * ce_f
    max_ent = np.log(V)
    gate_w1 = _f32(gate_w1); gate_b1 = _f32(gate_b1)
    gate_w2 = _f32(gate_w2); gate_b2 = _f32(gate_b2)
    for j in range(2):
        ce_j, pred_j, ent_j = ce_and_stats(e_logits[j])
        loss += WEIGHTS[j] * ce_j
        agree = (pred_j == pred_f).astype(np.float64).mean(-1, keepdims=True)
        gact = np.maximum(hm[j] @ gate_w1[j].T.astype(np.float64)
                          + gate_b1[j].astype(np.float64), 0.0)
        conf = 1.0 / (1.0 + np.exp(-(gact @ gate_w2[j].T.astype(np.float64)
                                     + gate_b2[j].astype(np.float64))))
        c = np.clip(conf, 1e-7, 1.0 - 1e-7)
        loss += 0.5 * float(-(agree * np.log(c)
                              + (1.0 - agree) * np.log(1.0 - c)).mean())
        overconf = (1.0 - ent_j / max_ent) * (pred_j != pred_f)
        loss += 0.1 * float(overconf.mean())

    return final_logits.astype(np.float32), np.float32(loss)


# revision 27
# speedup vs baseline: 1.0891x; 1.0891x over previous
"""Trainium2 Bass kernel for nn_AdaptiveDepthRWKV (8 NeuronCores, SPMD).

Sharding: token-parallel backbone (core r owns timesteps [128r,128(r+1)) of both
batches; activations feature-major [d, tok] in SBUF), weights replicated and
streamed bf16 from HBM with per-partition-contiguous layouts. The RWKV
decay-cumsum runs as a native DVE tensor_tensor_scan recurrence
z_t = dec*z_{t-1} + b_t*kv_t with a tiny carry all-gather across cores.
GroupNorm uses a linearity trick: the Wo_tm matmul runs on un-normalized data,
then an istd*P + cvec rank-1 fixup lands after the stats all-reduce. Heads are
vocab-sharded (4000 cols/core) after all-gathering the LN'd hidden state. Loss
terms are assembled on host in f64 from the full logits.
"""
import sys

sys.path.insert(0, "/opt/trn_rl_repo")

import numpy as np
import ml_dtypes

import concourse.bass as bass
import concourse.mybir as mybir
import concourse.tile as tile
from concourse import bacc
from concourse.bass_utils import run_bass_kernel_spmd

F32 = mybir.dt.float32
BF16 = mybir.dt.bfloat16
AF = mybir.ActivationFunctionType
OP = mybir.AluOpType

V, D, DFF, L, B, T = 32000, 1024, 4096, 12, 2, 1024
R = 8                 # cores
TL = T // R           # 128 timesteps per core
NT = B * TL           # 256 local tokens, free order: [b0 t0..127, b1 t0..127]
VS = V // R           # 4000 vocab cols per core
DC = D // 128         # 8 hidden chunks
FC = DFF // 128       # 32 ffn chunks
NV = 8                # vocab chunks per core
VC = VS // NV         # 500
EXITS = (4, 8)
WEIGHTS = (0.3, 0.5, 1.0)

_LN_COLS = {}


def _lncol(name):
    _LN_COLS[name] = len(_LN_COLS)


_lncol("in_w"); _lncol("in_b")
for _i in range(L):
    for _n in ("ln1_w", "ln1_b", "ln2_w", "ln2_b"):
        _lncol(f"{_n}_{_i}")
_lncol("out_w"); _lncol("out_b")
for _j in range(2):
    _lncol(f"ex_w_{_j}"); _lncol(f"ex_b_{_j}")
NLN = len(_LN_COLS)


def _build(n_layers=L, do_heads=True, dbg_x=False, stage=99):
    nc = bacc.Bacc("TRN2", target_bir_lowering=False, debug=False, num_devices=R)

    # ---------------- inputs ----------------
    xembT = nc.dram_tensor("xembT", [D, NT], F32, kind="ExternalInput")
    lnp_d = nc.dram_tensor("lnp", [128, DC, NLN], F32, kind="ExternalInput")
    decayT_d = nc.dram_tensor("decayT", [128, DC, L], F32, kind="ExternalInput")
    gnaux_d = nc.dram_tensor("gnaux", [128, DC, 2 * L], F32, kind="ExternalInput")
    iota_d = nc.dram_tensor("iota", [1, TL], F32, kind="ExternalInput")
    rmask_d = nc.dram_tensor("rmask", [1, R], F32, kind="ExternalInput")
    # weights repacked [out_hi, in_lo(128), in_hi, out_lo] (partition-contiguous)
    w_qkv = [[nc.dram_tensor(f"w{n}{i}", [DC, 128, DC, 128], BF16,
                             kind="ExternalInput")
              for n in ("r", "k", "v", "o")] for i in range(n_layers)]
    w_12 = [[nc.dram_tensor(f"wf{n}{i}", [FC, 128, DC, 128], BF16,
                            kind="ExternalInput")
             for n in (1, 2)] for i in range(n_layers)]
    w_cm = [nc.dram_tensor(f"wcm{i}", [DC, 128, FC, 128], BF16,
                           kind="ExternalInput") for i in range(n_layers)]
    headT = [nc.dram_tensor(f"headT{j}", [NV, 128, DC, VC], BF16,
                            kind="ExternalInput")
             for j in range(3 if do_heads else 0)]

    # ---------------- outputs ----------------
    out_logits = [nc.dram_tensor(n, [B * T, VS], F32, kind="ExternalOutput")
                  for n in ("exit0", "exit1", "logits")]
    hemean_o = nc.dram_tensor("hemean", [128, 4, DC], F32, kind="ExternalOutput")
    xdbg_o = (nc.dram_tensor("xdbg", [128, DC, NT], F32, kind="ExternalOutput")
              if dbg_x else None)

    # ---------------- collective buffers ----------------
    cc_carry_in = [nc.dram_tensor(f"cyi{i}", [128, 2 * DC], F32) for i in range(L)]
    cc_carry_out = [nc.dram_tensor(f"cyo{i}", [128 * R, 2 * DC], F32,
                                   addr_space="Shared") for i in range(L)]
    cc_gn_in = [nc.dram_tensor(f"gni{i}", [1, 4], F32) for i in range(L)]
    cc_gn_out = [nc.dram_tensor(f"gno{i}", [1, 4], F32, addr_space="Shared")
                 for i in range(L)]
    heb = [nc.dram_tensor(f"heb{j}", [128, DC * NT], BF16) for j in range(3)]
    heg = [nc.dram_tensor(f"heg{j}", [128 * R, DC * NT], BF16,
                          addr_space="Shared") for j in range(3)]
    RG = [list(range(R))]

    with tile.TileContext(nc) as tc, \
         tc.tile_pool(name="persist", bufs=1) as pers, \
         tc.tile_pool(name="act1", bufs=1) as act1, \
         tc.tile_pool(name="act2", bufs=2) as act2, \
         tc.tile_pool(name="scratch", bufs=2) as scr, \
         tc.tile_pool(name="small", bufs=2) as smp, \
         tc.tile_pool(name="wdd", bufs=2) as wddp, \
         tc.tile_pool(name="wcmp", bufs=2) as wcmp, \
         tc.tile_pool(name="whead", bufs=2) as wheadp, \
         tc.tile_pool(name="hegp", bufs=1) as hegp, \
         tc.tile_pool(name="ps1", bufs=1, space="PSUM") as pss, \
         tc.tile_pool(name="psa", bufs=2, space="PSUM") as psa, \
         tc.tile_pool(name="psb", bufs=2, space="PSUM") as psb, \
         tc.tile_pool(name="psd", bufs=2, space="PSUM") as psd:

        # ---- persistent setup ----
        x = pers.tile([128, DC, NT], F32)          # residual master, feature-major
        lnp = pers.tile([128, DC, NLN], F32)
        decayT = pers.tile([128, DC, L], F32)
        gnaux = pers.tile([128, DC, 2 * L], F32)
        iota_b = pers.tile([128, TL], F32)
        rmask_b = pers.tile([128, R], F32)
        ones_col = pers.tile([128, 1], F32)
        nc.sync.dma_start(lnp[:], lnp_d[:])
        nc.sync.dma_start(decayT[:], decayT_d[:])
        nc.sync.dma_start(gnaux[:], gnaux_d[:])
        iota_r = pers.tile([1, TL], F32)
        rmask_r = pers.tile([1, R], F32)
        nc.sync.dma_start(iota_r[:], iota_d[:])
        nc.sync.dma_start(rmask_r[:], rmask_d[:])
        nc.gpsimd.partition_broadcast(iota_b[:], iota_r[:])
        nc.gpsimd.partition_broadcast(rmask_b[:], rmask_r[:])
        nc.vector.memset(ones_col[:], 1.0)

        def lncol(name):
            return lnp[:, :, _LN_COLS[name]]

        # ---- feature-major layernorm ----
        def layer_norm(src, w_cols, b_cols, out_dtype, accum_cols=None,
                       out_tile=None):
            """src: [128, DC, NT] f32. Returns h [128, DC, NT] out_dtype."""
            sum_ps = pss.tile([1, NT], F32, tag="lnsum")
            sq_ps = pss.tile([1, NT], F32, tag="lnsq")
            for c in range(DC):
                sq = scr.tile([128, NT], F32, tag="lnsqt")
                nc.scalar.activation(sq[:], src[:, c, :], AF.Square)
                nc.tensor.matmul(sum_ps[:], ones_col[:], src[:, c, :],
                                 start=(c == 0), stop=(c == DC - 1))
                nc.tensor.matmul(sq_ps[:], ones_col[:], sq[:],
                                 start=(c == 0), stop=(c == DC - 1))
            mrow = smp.tile([1, NT], F32, tag="lnm")
            var = smp.tile([1, NT], F32, tag="lnv")
            msq = smp.tile([1, NT], F32, tag="lnmsq")
            nc.vector.tensor_scalar_mul(mrow[:], sum_ps[:], 1.0 / D)
            nc.vector.tensor_scalar_mul(var[:], sq_ps[:], 1.0 / D)
            nc.vector.tensor_tensor(msq[:], mrow[:], mrow[:], OP.mult)
            nc.vector.tensor_tensor(var[:], var[:], msq[:], OP.subtract)
            nc.vector.tensor_scalar_add(var[:], var[:], 1e-5)
            nc.scalar.sqrt(var[:], var[:])
            nc.vector.reciprocal(var[:], var[:])          # istd row
            nc.vector.tensor_tensor(mrow[:], mrow[:], var[:], OP.mult)  # m*istd
            ib = smp.tile([128, NT], F32, tag="lnib")
            mib = smp.tile([128, NT], F32, tag="lnmib")
            nc.gpsimd.partition_broadcast(ib[:], var[:])
            nc.gpsimd.partition_broadcast(mib[:], mrow[:])
            if out_tile is None:
                h = act2.tile([128, DC, NT], out_dtype, tag="h")
            else:
                h = out_tile
            for c in range(DC):
                t1 = scr.tile([128, NT], F32, tag="lnt1")
                nc.vector.tensor_tensor(t1[:], src[:, c, :], ib[:], OP.mult)
                nc.vector.tensor_tensor(t1[:], t1[:], mib[:], OP.subtract)
                if accum_cols is None:
                    nc.vector.tensor_scalar(h[:, c, :], t1[:],
                                            w_cols[:, c:c + 1], b_cols[:, c:c + 1],
                                            OP.mult, OP.add)
                else:
                    for b in range(B):
                        sl = slice(b * TL, (b + 1) * TL)
                        nc.vector.tensor_scalar(
                            h[:, c, sl], t1[:, sl],
                            w_cols[:, c:c + 1], b_cols[:, c:c + 1],
                            OP.mult, OP.add,
                            accum_out=accum_cols[b][:, c:c + 1])
            return h

        # ---- input embedding LN -> x ----
        xemb = act1.tile([128, DC, NT], F32, tag="kv")   # reuse kv slot
        nc.sync.dma_start(xemb[:], xembT.rearrange("(o p) f -> p o f", p=128))
        layer_norm(xemb, lncol("in_w"), lncol("in_b"), F32, out_tile=x)

        def dump(t):
            if xdbg_o is not None:
                nc.sync.dma_start(xdbg_o[:], t[:])

        # ---- deferred vocab-sharded heads: LN+AG now, matmuls drip-fed ----
        pending = []
        heg_sb_cache = {}

        def head_pre(j):
            if j < 2:
                wc, bc = lncol(f"ex_w_{j}"), lncol(f"ex_b_{j}")
                hm = [smp.tile([128, DC], F32, tag=f"hm{j}{b}", name="hm")
                      for b in range(B)]
                for b in range(B):
                    nc.vector.memset(hm[b][:], 0.0)
                he = layer_norm(x, wc, bc, BF16, accum_cols=hm)
                for b in range(B):
                    nc.sync.dma_start(hemean_o[:, j * 2 + b, :], hm[b][:])
            else:
                he = layer_norm(x, lncol("out_w"), lncol("out_b"), BF16)
            nc.sync.dma_start(heb[j].rearrange("p (o f) -> p o f", o=DC), he[:])
            nc.gpsimd.collective_compute(
                "AllGather", OP.bypass, replica_groups=RG,
                ins=[heb[j].ap().opt()], outs=[heg[j].ap().opt()])

        def queue_head_blocks(j):
            for vc in range(NV):
                def block(j=j, vc=vc):
                    if j not in heg_sb_cache:
                        hs = hegp.tile([128, R, DC, NT], BF16, tag="heg",
                                       name="heg_sb")
                        nc.sync.dma_start(
                            hs[:], heg[j].rearrange("(r p) (o f) -> p r o f",
                                                    p=128, o=DC))
                        heg_sb_cache.clear()
                        heg_sb_cache[j] = hs
                    hs = heg_sb_cache[j]
                    wh = wheadp.tile([128, DC, VC], BF16, tag="wh", name="wh")
                    nc.sync.dma_start(wh[:], headT[j][vc])
                    for mt in range(2 * R):
                        rb, jh = mt // 2, mt % 2
                        lps = psa.tile([128, VC], F32, tag="pa", name="lps")
                        for d in range(DC):
                            nc.tensor.matmul(
                                lps[:], hs[:, rb, d, jh * 128:(jh + 1) * 128],
                                wh[:, d, :], start=(d == 0), stop=(d == DC - 1))
                        ls = scr.tile([128, VC], F32, tag="lsb", name="ls")
                        nc.scalar.copy(ls[:], lps[:])
                        nc.sync.dma_start(
                            out_logits[j][mt * 128:(mt + 1) * 128,
                                          vc * VC:(vc + 1) * VC], ls[:])
                pending.append(block)

        def drain_blocks(k):
            n = 0
            while pending and n < k:
                pending.pop(0)()
                n += 1

        # =================================================================
        for i in range(n_layers):
            # ---------------- TimeMix ----------------
            h = layer_norm(x, lncol(f"ln1_w_{i}"), lncol(f"ln1_b_{i}"), BF16)
            if stage == 0:
                hh = act1.tile([128, DC, NT], F32, tag="kv")
                for c in range(DC):
                    nc.vector.tensor_copy(hh[:, c, :], h[:, c, :])
                dump(hh)
                break

            # decay tables; w = min(exp(-t*lndec), 1e10), b = min(scale*1e10, 1)
            dec = smp.tile([128, DC], F32, tag="dec")
            lndec = smp.tile([128, DC], F32, tag="lndec")
            nc.scalar.activation(dec[:], decayT[:, :, i], AF.Sigmoid)
            nc.vector.tensor_scalar_max(dec[:], dec[:], 1e-7)
            nc.scalar.activation(lndec[:], dec[:], AF.Ln)
            scale_t = act1.tile([128, DC, TL], F32, tag="scale")
            for c in range(DC):
                sc0 = scr.tile([128, TL], F32, tag="sc0")
                nc.vector.tensor_scalar_mul(sc0[:], iota_b[:], lndec[:, c:c + 1])
                nc.scalar.activation(scale_t[:, c, :], sc0[:], AF.Exp)

            # k,v projections + kv product + carry sums
            kv = act1.tile([128, DC, NT], F32, tag="kv")
            r = act1.tile([128, DC, NT], F32, tag="r")
            ysum = smp.tile([128, 2 * DC], F32, tag="ysum")
            wkq = wvq = wrq = None
            for c in range(DC):
                if c % 2 == 0:
                    wkq = wddp.tile([128, 2, DC, 128], BF16, tag="wk", name="wk")
                    wvq = wddp.tile([128, 2, DC, 128], BF16, tag="wv", name="wv")
                    nc.sync.dma_start(wkq[:], w_qkv[i][1][c // 2 * 2:c // 2 * 2 + 2]
                                      .rearrange("q p o f -> p q o f"))
                    nc.sync.dma_start(wvq[:], w_qkv[i][2][c // 2 * 2:c // 2 * 2 + 2]
                                      .rearrange("q p o f -> p q o f"))
                ci = c % 2
                kps = psa.tile([128, NT], F32, tag="pa")
                vps = psb.tile([128, NT], F32, tag="pb")
                for d in range(DC):
                    nc.tensor.matmul(kps[:], wkq[:, ci, d, :], h[:, d, :],
                                     start=(d == 0), stop=(d == DC - 1))
                for d in range(DC):
                    nc.tensor.matmul(vps[:], wvq[:, ci, d, :], h[:, d, :],
                                     start=(d == 0), stop=(d == DC - 1))
                ksb = scr.tile([128, NT], F32, tag="ksb")
                nc.scalar.copy(ksb[:], kps[:])
                nc.vector.tensor_tensor(kv[:, c, :], ksb[:], vps[:], OP.mult)
                wtab = scr.tile([128, TL], F32, tag="wtab")
                nc.vector.tensor_scalar(wtab[:], iota_b[:], lndec[:, c:c + 1],
                                        -1.0, OP.mult, OP.mult)
                nc.scalar.activation(wtab[:], wtab[:], AF.Exp)
                nc.vector.tensor_scalar_min(wtab[:], wtab[:], 1e10)
                for b in range(B):
                    sc = scr.tile([128, TL], F32, tag="ysc")
                    nc.vector.tensor_tensor(sc[:], kv[:, c, b * TL:(b + 1) * TL],
                                            wtab[:], OP.mult)
                    nc.vector.tensor_reduce(
                        ysum[:, c * 2 + b:c * 2 + b + 1], sc[:],
                        axis=mybir.AxisListType.X, op=OP.add)
            if stage == 1:
                dump(kv)
                break
            nc.sync.dma_start(cc_carry_in[i][:], ysum[:])
            nc.gpsimd.collective_compute(
                "AllGather", OP.bypass, replica_groups=RG,
                ins=[cc_carry_in[i].ap().opt()], outs=[cc_carry_out[i].ap().opt()])

            for c in range(DC):
                if c % 2 == 0:
                    wrq = wddp.tile([128, 2, DC, 128], BF16, tag="wr", name="wr")
                    nc.sync.dma_start(wrq[:], w_qkv[i][0][c // 2 * 2:c // 2 * 2 + 2]
                                      .rearrange("q p o f -> p q o f"))
                rps = psa.tile([128, NT], F32, tag="pa")
                for d in range(DC):
                    nc.tensor.matmul(rps[:], wrq[:, c % 2, d, :], h[:, d, :],
                                     start=(d == 0), stop=(d == DC - 1))
                nc.scalar.activation(r[:, c, :], rps[:], AF.Sigmoid)
            if stage > 90:
                drain_blocks(1)

            # local scan (z written in place over kv)
            z = kv
            for c in range(DC):
                btab = scr.tile([128, TL], F32, tag="btab")
                nc.vector.tensor_scalar(btab[:], scale_t[:, c, :], 1e10, 1.0,
                                        OP.mult, OP.min)
                for b in range(B):
                    sl = slice(b * TL, (b + 1) * TL)
                    bkv = scr.tile([128, TL], F32, tag="bkv")
                    nc.vector.tensor_tensor(bkv[:], kv[:, c, sl], btab[:],
                                            OP.mult)
                    nc.vector.tensor_tensor_scan(
                        z[:, c, sl], dec[:, c:c + 1].to_broadcast((128, TL)),
                        bkv[:], 0.0, OP.mult, OP.add)
            if stage == 2:
                dump(z)
                break

            # carry prefix
            carry_all = smp.tile([128, R, 2 * DC], F32, tag="carry_all")
            nc.sync.dma_start(carry_all[:],
                              cc_carry_out[i].rearrange("(r p) f -> p r f", p=128))
            cmask = smp.tile([128, R, 2 * DC], F32, tag="cmask")
            nc.vector.tensor_tensor(
                cmask[:], carry_all[:],
                rmask_b[:, :, None].to_broadcast((128, R, 2 * DC)), OP.mult)
            cprev = smp.tile([128, 2 * DC], F32, tag="cprev")
            nc.vector.tensor_reduce(cprev[:],
                                    cmask[:].rearrange("p r f -> p f r"),
                                    axis=mybir.AxisListType.X, op=OP.add)

            # out = r * (scale*cprev + z); stats + bf16 cast (all DVE)
            out_bf = act1.tile([128, DC, NT], BF16, tag="out_bf")
            ssum = smp.tile([128, 2 * DC], F32, tag="ssum")
            ssq = smp.tile([128, 2 * DC], F32, tag="ssq")
            nc.vector.memset(ssum[:], 0.0)
            nc.vector.memset(ssq[:], 0.0)
            for c in range(DC):
                of = scr.tile([128, NT], F32, tag="outf")
                for b in range(B):
                    sl = slice(b * TL, (b + 1) * TL)
                    j = c * 2 + b
                    nc.vector.scalar_tensor_tensor(
                        of[:, sl], scale_t[:, c, :], cprev[:, j:j + 1],
                        z[:, c, sl], OP.mult, OP.add)
                    nc.vector.tensor_tensor(of[:, sl], r[:, c, sl], of[:, sl],
                                            OP.mult)
                    nc.scalar.activation(out_bf[:, c, sl], of[:, sl], AF.Copy,
                                         accum_out=ssum[:, j:j + 1])
                    sq2 = scr.tile([128, TL], F32, tag="gnsq")
                    nc.vector.scalar_tensor_tensor(
                        sq2[:], of[:, sl], 1.0, of[:, sl], OP.mult, OP.mult,
                        accum_out=ssq[:, j:j + 1])
            if stage == 3:
                oo = act1.tile([128, DC, NT], F32, tag="kv")
                for c in range(DC):
                    nc.vector.tensor_copy(oo[:, c, :], out_bf[:, c, :])
                dump(oo)
                break

            # GN stats partition-reduce + AllReduce
            sgn_ps = pss.tile([1, NT], F32, tag="lnsum")
            sqn_ps = pss.tile([1, NT], F32, tag="lnsq")
            nc.tensor.matmul(sgn_ps[:, :2 * DC], ones_col[:], ssum[:])
            nc.tensor.matmul(sqn_ps[:, :2 * DC], ones_col[:], ssq[:])
            st_row = smp.tile([1, 4], F32, tag="gnrow")
            nc.vector.tensor_reduce(
                st_row[:, 0:2],
                sgn_ps[:, :2 * DC].rearrange("p (c b) -> p b c", b=B),
                axis=mybir.AxisListType.X, op=OP.add)
            nc.vector.tensor_reduce(
                st_row[:, 2:4],
                sqn_ps[:, :2 * DC].rearrange("p (c b) -> p b c", b=B),
                axis=mybir.AxisListType.X, op=OP.add)
            nc.sync.dma_start(cc_gn_in[i][:], st_row[:])
            nc.gpsimd.collective_compute(
                "AllReduce", OP.add, replica_groups=RG,
                ins=[cc_gn_in[i].ap().opt()], outs=[cc_gn_out[i].ap().opt()])

            # finish GN stats: istd / -m*istd per batch
            g_st = smp.tile([1, 4], F32, tag="gst")
            nc.sync.dma_start(g_st[:], cc_gn_out[i][:])
            inv = 1.0 / (T * D)
            mrow = smp.tile([1, 2], F32, tag="gm")
            vrow = smp.tile([1, 2], F32, tag="gv")
            t2 = smp.tile([1, 2], F32, tag="gt2")
            nc.vector.tensor_scalar_mul(mrow[:], g_st[:, 0:2], inv)
            nc.vector.tensor_scalar_mul(vrow[:], g_st[:, 2:4], inv)
            nc.vector.tensor_tensor(t2[:], mrow[:], mrow[:], OP.mult)
            nc.vector.tensor_tensor(vrow[:], vrow[:], t2[:], OP.subtract)
            nc.vector.tensor_scalar_add(vrow[:], vrow[:], 1e-5)
            nc.scalar.sqrt(vrow[:], vrow[:])
            nc.vector.reciprocal(vrow[:], vrow[:])           # istd [1,2]
            nc.vector.tensor_tensor(mrow[:], mrow[:], vrow[:], OP.mult)
            nc.vector.tensor_scalar_mul(mrow[:], mrow[:], -1.0)  # -m*istd
            row4 = smp.tile([1, 4], F32, tag="grow4")
            nc.vector.tensor_copy(row4[:, 0:2], vrow[:])
            nc.vector.tensor_copy(row4[:, 2:4], mrow[:])
            bc4 = smp.tile([128, 4], F32, tag="gbc4")
            nc.gpsimd.partition_broadcast(bc4[:], row4[:])
            cvec = smp.tile([128, DC, B], F32, tag="cvec")
            for b in range(B):
                nc.vector.tensor_scalar_mul(cvec[:, :, b], gnaux[:, :, 2 * i],
                                            bc4[:, 2 + b:3 + b])
                nc.vector.tensor_tensor(cvec[:, :, b], cvec[:, :, b],
                                        gnaux[:, :, 2 * i + 1], OP.add)

            # P = out_bf @ WoT (gn_w folded); fixup; residual add
            woq = None
            for d in range(DC):
                if d % 2 == 0:
                    woq = wddp.tile([128, 2, DC, 128], BF16, tag="wo", name="wo")
                    nc.sync.dma_start(woq[:], w_qkv[i][3][d // 2 * 2:d // 2 * 2 + 2]
                                      .rearrange("q p o f -> p q o f"))
                P = psd.tile([128, NT], F32, tag="pd")
                for c in range(DC):
                    nc.tensor.matmul(P[:], woq[:, d % 2, c, :], out_bf[:, c, :],
                                     start=(c == 0), stop=(c == DC - 1))
                for b in range(B):
                    sl = slice(b * TL, (b + 1) * TL)
                    nc.vector.tensor_scalar(P[:, sl], P[:, sl],
                                            bc4[:, b:b + 1], cvec[:, d, b:b + 1],
                                            OP.mult, OP.add)
                nc.vector.tensor_tensor(x[:, d, :], x[:, d, :], P[:], OP.add)
            if stage > 90:
                drain_blocks(2)
            if stage == 4:
                dump(x)
                break

            # ---------------- ChannelMix ----------------
            h2 = layer_norm(x, lncol(f"ln2_w_{i}"), lncol(f"ln2_b_{i}"), BF16)
            for gh in range(2):                       # ffn halves
                g = act1.tile([128, FC // 2, NT], BF16, tag="g", name="g")
                for fg in range(4):                   # groups of 4 f-tiles
                    ft0 = gh * (FC // 2) + fg * 4
                    w1c = wddp.tile([128, 4, DC, 128], BF16, tag="w1", name="w1")
                    w2c = wddp.tile([128, 4, DC, 128], BF16, tag="w2", name="w2")
                    nc.sync.dma_start(w1c[:], w_12[i][0][ft0:ft0 + 4]
                                      .rearrange("q p o f -> p q o f"))
                    nc.sync.dma_start(w2c[:], w_12[i][1][ft0:ft0 + 4]
                                      .rearrange("q p o f -> p q o f"))
                    for fi in range(4):
                        aps_ = psa.tile([128, NT], F32, tag="pa")
                        bps_ = psb.tile([128, NT], F32, tag="pb")
                        for d in range(DC):
                            nc.tensor.matmul(aps_[:], w1c[:, fi, d, :],
                                             h2[:, d, :],
                                             start=(d == 0), stop=(d == DC - 1))
                        for d in range(DC):
                            nc.tensor.matmul(bps_[:], w2c[:, fi, d, :],
                                             h2[:, d, :],
                                             start=(d == 0), stop=(d == DC - 1))
                        sa = scr.tile([128, NT], F32, tag="silu")
                        nc.scalar.activation(sa[:], aps_[:], AF.Silu)
                        nc.vector.tensor_tensor(g[:, fg * 4 + fi, :], sa[:],
                                                bps_[:], OP.mult)
                for d in range(DC):
                    wcq = wcmp.tile([128, FC // 2, 128], BF16, tag="wcq")
                    nc.sync.dma_start(
                        wcq[:], w_cm[i][d, :, gh * (FC // 2):(gh + 1) * (FC // 2), :])
                    d2 = psd.tile([128, NT], F32, tag="pd")
                    for fh in range(FC // 2):
                        nc.tensor.matmul(d2[:], wcq[:, fh, :], g[:, fh, :],
                                         start=(fh == 0), stop=(fh == FC // 2 - 1))
                    nc.vector.tensor_tensor(x[:, d, :], x[:, d, :], d2[:], OP.add)

            # ---------------- early exits ----------------
            if do_heads and (i + 1) in EXITS:
                j = EXITS.index(i + 1)
                head_pre(j)
                queue_head_blocks(j)

        # ---------------- final head ----------------
        if do_heads:
            head_pre(2)
            queue_head_blocks(2)
            drain_blocks(len(pending))
        if xdbg_o is not None and stage > 90:
            nc.sync.dma_start(xdbg_o[:], x[:])

    nc.compile()
    return nc


# =====================================================================
# host side
# =====================================================================
_CACHE = {}


def _get_nc():
    if "nc" not in _CACHE:
        _CACHE["nc"] = _build()
    return _CACHE["nc"]


def _bf(a):
    return np.ascontiguousarray(a.astype(ml_dtypes.bfloat16))


def _f32(a):
    return np.ascontiguousarray(np.asarray(a, dtype=np.float32))


def _featmaj(vec):
    """[D] -> [128, DC] feature-major column block."""
    return np.ascontiguousarray(vec.reshape(DC, 128).T)


def _repack(Wout_in):
    """W [n_out, n_in] -> [out_hi, in_lo(128), in_hi, out_lo(128)] bf16."""
    no, ni = Wout_in.shape
    a = Wout_in.reshape(no // 128, 128, ni // 128, 128)
    return _bf(a.transpose(0, 3, 2, 1))


def _repack_cm(Wdf):
    """Wo_cm [D, DFF] -> [d_hi, f_lo(128), f_hi, d_lo(128)] bf16."""
    a = Wdf.reshape(DC, 128, FC, 128)
    return _bf(a.transpose(0, 3, 2, 1))


def _repack_head(Wvd):
    """W [VS, D] -> [NV, d_lo(128), DC, VC] bf16."""
    a = Wvd.reshape(NV, VC, DC, 128)
    return _bf(a.transpose(0, 3, 2, 1))


def kernel(idx, targets, embed, ln_in_w, ln_in_b, Wr, Wk, Wv, Wo_tm, decay,
           gn_w, gn_b, ln1_w, ln1_b, ln2_w, ln2_b, W1, W2, Wo_cm,
           ln_out_w, ln_out_b, exit_ln_w, exit_ln_b, exit_head,
           gate_w1, gate_b1, gate_w2, gate_b2):
    idx = np.asarray(idx)
    targets = np.asarray(targets)
    embed = _f32(embed)
    nc = _get_nc()

    # ---- shared (rank-independent) prep ----
    lnp = np.zeros((128, DC, NLN), np.float32)

    def setln(name, vec):
        lnp[:, :, _LN_COLS[name]] = _featmaj(_f32(vec))

    setln("in_w", ln_in_w); setln("in_b", ln_in_b)
    for i in range(L):
        setln(f"ln1_w_{i}", ln1_w[i]); setln(f"ln1_b_{i}", ln1_b[i])
        setln(f"ln2_w_{i}", ln2_w[i]); setln(f"ln2_b_{i}", ln2_b[i])
    setln("out_w", ln_out_w); setln("out_b", ln_out_b)
    for j in range(2):
        setln(f"ex_w_{j}", exit_ln_w[j]); setln(f"ex_b_{j}", exit_ln_b[j])

    decay = _f32(decay)
    decayT = np.ascontiguousarray(
        np.stack([_featmaj(decay[i]) for i in range(L)], axis=-1))

    gnaux = np.zeros((128, DC, 2 * L), np.float32)
    Wo_tm = _f32(Wo_tm); gn_w = _f32(gn_w); gn_b = _f32(gn_b)
    for i in range(L):
        gnaux[:, :, 2 * i] = _featmaj(Wo_tm[i] @ gn_w[i])
        gnaux[:, :, 2 * i + 1] = _featmaj(Wo_tm[i] @ gn_b[i])

    shared = {"lnp": lnp, "decayT": decayT, "gnaux": gnaux}
    for i in range(L):
        shared[f"wr{i}"] = _repack(_f32(Wr[i]))
        shared[f"wk{i}"] = _repack(_f32(Wk[i]))
        shared[f"wv{i}"] = _repack(_f32(Wv[i]))
        shared[f"wo{i}"] = _repack(Wo_tm[i] * gn_w[i][None, :])
        shared[f"wf1{i}"] = _repack(_f32(W1[i]))
        shared[f"wf2{i}"] = _repack(_f32(W2[i]))
        shared[f"wcm{i}"] = _repack_cm(_f32(Wo_cm[i]))

    x_emb = embed[np.asarray(idx, dtype=np.int64)]        # [B, T, D]
    exit_head = _f32(exit_head)

    in_maps = []
    for r in range(R):
        m = dict(shared)
        sl = x_emb[:, r * TL:(r + 1) * TL, :]              # [B, TL, D]
        m["xembT"] = np.ascontiguousarray(
            sl.transpose(2, 0, 1).reshape(D, NT).astype(np.float32))
        m["iota"] = (np.arange(TL, dtype=np.float32) + r * TL).reshape(1, TL)
        m["rmask"] = (np.arange(R) < r).astype(np.float32).reshape(1, R)
        vs = slice(r * VS, (r + 1) * VS)
        m["headT0"] = _repack_head(exit_head[0, vs, :])
        m["headT1"] = _repack_head(exit_head[1, vs, :])
        m["headT2"] = _repack_head(embed[vs, :])
        in_maps.append(m)

    _CACHE["in_maps"] = in_maps
    res = run_bass_kernel_spmd(nc, in_maps, core_ids=list(range(R)))

    # ---- unshard ----
    def assemble(name):
        shards = []
        for r in range(R):
            a = res.results[r][name]                       # [2048, VS]
            a = a.reshape(R, B, TL, VS).transpose(1, 0, 2, 3).reshape(B, T, VS)
            shards.append(a)
        return np.concatenate(shards, axis=-1)             # [B, T, V]

    final_logits = assemble("logits")
    e_logits = [assemble("exit0"), assemble("exit1")]

    hm = np.zeros((2, B, D), np.float64)
    for r in range(R):
        a = res.results[r]["hemean"]                        # [128, 4, DC]
        for j in range(2):
            for b in range(B):
                hm[j, b] += a[:, j * 2 + b, :].T.reshape(D).astype(np.float64)
    hm /= T

    # ---- loss in f64 ----
    tgt = np.asarray(targets, dtype=np.int64)

    def ce_and_stats(lg):
        lg = lg.astype(np.float64)
        mx = lg.max(-1, keepdims=True)
        ex = np.exp(lg - mx)
        Z = ex.sum(-1, keepdims=True)
        lse = (mx + np.log(Z))[..., 0]                     # [B,T]
        tl_ = np.take_along_axis(lg, tgt[..., None], -1)[..., 0]
        ce = float((lse - tl_).mean())
        pred = lg.argmax(-1)
        p = ex / Z
        ent = lse - (p * lg).sum(-1)                       # [B,T]
        return ce, pred, ent

    ce_f, pred_f, _ = ce_and_stats(final_logits)
    loss = WEIGHTS[-1] * ce_f
    max_ent = np.log(V)
    gate_w1 = _f32(gate_w1); gate_b1 = _f32(gate_b1)
    gate_w2 = _f32(gate_w2); gate_b2 = _f32(gate_b2)
    for j in range(2):
        ce_j, pred_j, ent_j = ce_and_stats(e_logits[j])
        loss += WEIGHTS[j] * ce_j
        agree = (pred_j == pred_f).astype(np.float64).mean(-1, keepdims=True)
        gact = np.maximum(hm[j] @ gate_w1[j].T.astype(np.float64)
                          + gate_b1[j].astype(np.float64), 0.0)
        conf = 1.0 / (1.0 + np.exp(-(gact @ gate_w2[j].T.astype(np.float64)
                                     + gate_b2[j].astype(np.float64))))
        c = np.clip(conf, 1e-7, 1.0 - 1e-7)
        loss += 0.5 * float(-(agree * np.log(c)
                              + (1.0 - agree) * np.log(1.0 - c)).mean())
        overconf = (1.0 - ent_j / max_ent) * (pred_j != pred_f)
        loss += 0.1 * float(overconf.mean())

    return final_logits.astype(np.float32), np.float32(loss)


# revision 28
# speedup vs baseline: 1.1133x; 1.0222x over previous
"""Trainium2 Bass kernel for nn_AdaptiveDepthRWKV (8 NeuronCores, SPMD).

Sharding: token-parallel backbone (core r owns timesteps [128r,128(r+1)) of both
batches; activations feature-major [d, tok] in SBUF), weights replicated and
streamed bf16 from HBM with per-partition-contiguous layouts. The RWKV
decay-cumsum runs as a native DVE tensor_tensor_scan recurrence
z_t = dec*z_{t-1} + b_t*kv_t with a tiny carry all-gather across cores.
GroupNorm uses a linearity trick: the Wo_tm matmul runs on un-normalized data,
then an istd*P + cvec rank-1 fixup lands after the stats all-reduce. Heads are
vocab-sharded (4000 cols/core) after all-gathering the LN'd hidden state. Loss
terms are assembled on host in f64 from the full logits.
"""
import sys

sys.path.insert(0, "/opt/trn_rl_repo")

import numpy as np
import ml_dtypes

import concourse.bass as bass
import concourse.mybir as mybir
import concourse.tile as tile
from concourse import bacc
from concourse.bass_utils import run_bass_kernel_spmd

F32 = mybir.dt.float32
BF16 = mybir.dt.bfloat16
AF = mybir.ActivationFunctionType
OP = mybir.AluOpType

V, D, DFF, L, B, T = 32000, 1024, 4096, 12, 2, 1024
R = 8                 # cores
TL = T // R           # 128 timesteps per core
NT = B * TL           # 256 local tokens, free order: [b0 t0..127, b1 t0..127]
VS = V // R           # 4000 vocab cols per core
DC = D // 128         # 8 hidden chunks
FC = DFF // 128       # 32 ffn chunks
NV = 8                # vocab chunks per core
VC = VS // NV         # 500
EXITS = (4, 8)
WEIGHTS = (0.3, 0.5, 1.0)

_LN_COLS = {}


def _lncol(name):
    _LN_COLS[name] = len(_LN_COLS)


_lncol("in_w"); _lncol("in_b")
for _i in range(L):
    for _n in ("ln1_w", "ln1_b", "ln2_w", "ln2_b"):
        _lncol(f"{_n}_{_i}")
_lncol("out_w"); _lncol("out_b")
for _j in range(2):
    _lncol(f"ex_w_{_j}"); _lncol(f"ex_b_{_j}")
NLN = len(_LN_COLS)


def _build(n_layers=L, do_heads=True, dbg_x=False, stage=99):
    nc = bacc.Bacc("TRN2", target_bir_lowering=False, debug=False, num_devices=R)

    # ---------------- inputs ----------------
    xembT = nc.dram_tensor("xembT", [D, NT], F32, kind="ExternalInput")
    lnp_d = nc.dram_tensor("lnp", [128, DC, NLN], F32, kind="ExternalInput")
    decayT_d = nc.dram_tensor("decayT", [128, DC, L], F32, kind="ExternalInput")
    gnaux_d = nc.dram_tensor("gnaux", [128, DC, 2 * L], F32, kind="ExternalInput")
    iota_d = nc.dram_tensor("iota", [1, TL], F32, kind="ExternalInput")
    rmask_d = nc.dram_tensor("rmask", [1, R], F32, kind="ExternalInput")
    # weights repacked [out_hi, in_lo(128), in_hi, out_lo] (partition-contiguous)
    w_qkv = [[nc.dram_tensor(f"w{n}{i}", [DC, 128, DC, 128], BF16,
                             kind="ExternalInput")
              for n in ("r", "k", "v", "o")] for i in range(n_layers)]
    w_12 = [[nc.dram_tensor(f"wf{n}{i}", [FC, 128, DC, 128], BF16,
                            kind="ExternalInput")
             for n in (1, 2)] for i in range(n_layers)]
    w_cm = [nc.dram_tensor(f"wcm{i}", [DC, 128, FC, 128], BF16,
                           kind="ExternalInput") for i in range(n_layers)]
    headT = [nc.dram_tensor(f"headT{j}", [NV, 128, DC, VC], BF16,
                            kind="ExternalInput")
             for j in range(3 if do_heads else 0)]

    # ---------------- outputs ----------------
    out_logits = [nc.dram_tensor(n, [B * T, VS], F32, kind="ExternalOutput")
                  for n in ("exit0", "exit1", "logits")]
    hemean_o = nc.dram_tensor("hemean", [128, 4, DC], F32, kind="ExternalOutput")
    xdbg_o = (nc.dram_tensor("xdbg", [128, DC, NT], F32, kind="ExternalOutput")
              if dbg_x else None)

    # ---------------- collective buffers ----------------
    cc_carry_in = [nc.dram_tensor(f"cyi{i}", [128, 2 * DC], F32) for i in range(L)]
    cc_carry_out = [nc.dram_tensor(f"cyo{i}", [128 * R, 2 * DC], F32,
                                   addr_space="Shared") for i in range(L)]
    cc_gn_in = [nc.dram_tensor(f"gni{i}", [1, 4], F32) for i in range(L)]
    cc_gn_out = [nc.dram_tensor(f"gno{i}", [1, 4], F32, addr_space="Shared")
                 for i in range(L)]
    heb = [nc.dram_tensor(f"heb{j}", [128, DC * NT], BF16) for j in range(3)]
    heg = [nc.dram_tensor(f"heg{j}", [128 * R, DC * NT], BF16,
                          addr_space="Shared") for j in range(3)]
    RG = [list(range(R))]

    with tile.TileContext(nc) as tc, \
         tc.tile_pool(name="persist", bufs=1) as pers, \
         tc.tile_pool(name="act1", bufs=1) as act1, \
         tc.tile_pool(name="act2", bufs=2) as act2, \
         tc.tile_pool(name="scratch", bufs=2) as scr, \
         tc.tile_pool(name="small", bufs=2) as smp, \
         tc.tile_pool(name="wdd", bufs=2) as wddp, \
         tc.tile_pool(name="wcmp", bufs=2) as wcmp, \
         tc.tile_pool(name="whead", bufs=2) as wheadp, \
         tc.tile_pool(name="hegp", bufs=1) as hegp, \
         tc.tile_pool(name="psa", bufs=3, space="PSUM") as psa, \
         tc.tile_pool(name="psb", bufs=3, space="PSUM") as psb, \
         tc.tile_pool(name="psd", bufs=2, space="PSUM") as psd:

        # ---- persistent setup ----
        x = pers.tile([128, DC, NT], F32)          # residual master, feature-major
        lnp = pers.tile([128, DC, NLN], F32)
        decayT = pers.tile([128, DC, L], F32)
        gnaux = pers.tile([128, DC, 2 * L], F32)
        iota_b = pers.tile([128, TL], F32)
        rmask_b = pers.tile([128, R], F32)
        ones_col = pers.tile([128, 1], F32)
        nc.sync.dma_start(lnp[:], lnp_d[:])
        nc.sync.dma_start(decayT[:], decayT_d[:])
        nc.sync.dma_start(gnaux[:], gnaux_d[:])
        iota_r = pers.tile([1, TL], F32)
        rmask_r = pers.tile([1, R], F32)
        nc.sync.dma_start(iota_r[:], iota_d[:])
        nc.sync.dma_start(rmask_r[:], rmask_d[:])
        nc.gpsimd.partition_broadcast(iota_b[:], iota_r[:])
        nc.gpsimd.partition_broadcast(rmask_b[:], rmask_r[:])
        nc.vector.memset(ones_col[:], 1.0)

        def lncol(name):
            return lnp[:, :, _LN_COLS[name]]

        # ---- feature-major layernorm ----
        def layer_norm(src, w_cols, b_cols, out_dtype, accum_cols=None,
                       out_tile=None):
            """src: [128, DC, NT] f32. Returns h [128, DC, NT] out_dtype."""
            sum_ps = psa.tile([1, NT], F32, tag="pa", name="sum_ps")
            sq_ps = psb.tile([1, NT], F32, tag="pb", name="sq_ps")
            for c in range(DC):
                sq = scr.tile([128, NT], F32, tag="lnsqt")
                nc.scalar.activation(sq[:], src[:, c, :], AF.Square)
                nc.tensor.matmul(sum_ps[:], ones_col[:], src[:, c, :],
                                 start=(c == 0), stop=(c == DC - 1))
                nc.tensor.matmul(sq_ps[:], ones_col[:], sq[:],
                                 start=(c == 0), stop=(c == DC - 1))
            mrow = smp.tile([1, NT], F32, tag="lnm")
            var = smp.tile([1, NT], F32, tag="lnv")
            msq = smp.tile([1, NT], F32, tag="lnmsq")
            nc.vector.tensor_scalar_mul(mrow[:], sum_ps[:], 1.0 / D)
            nc.vector.tensor_scalar_mul(var[:], sq_ps[:], 1.0 / D)
            nc.vector.tensor_tensor(msq[:], mrow[:], mrow[:], OP.mult)
            nc.vector.tensor_tensor(var[:], var[:], msq[:], OP.subtract)
            nc.vector.tensor_scalar_add(var[:], var[:], 1e-5)
            nc.scalar.sqrt(var[:], var[:])
            nc.vector.reciprocal(var[:], var[:])          # istd row
            nc.vector.tensor_tensor(mrow[:], mrow[:], var[:], OP.mult)  # m*istd
            ib = smp.tile([128, NT], F32, tag="lnib")
            mib = smp.tile([128, NT], F32, tag="lnmib")
            nc.gpsimd.partition_broadcast(ib[:], var[:])
            nc.gpsimd.partition_broadcast(mib[:], mrow[:])
            if out_tile is None:
                h = act2.tile([128, DC, NT], out_dtype, tag="h")
            else:
                h = out_tile
            for c in range(DC):
                t1 = scr.tile([128, NT], F32, tag="lnt1")
                nc.vector.tensor_tensor(t1[:], src[:, c, :], ib[:], OP.mult)
                nc.vector.tensor_tensor(t1[:], t1[:], mib[:], OP.subtract)
                if accum_cols is None:
                    nc.vector.tensor_scalar(h[:, c, :], t1[:],
                                            w_cols[:, c:c + 1], b_cols[:, c:c + 1],
                                            OP.mult, OP.add)
                else:
                    for b in range(B):
                        sl = slice(b * TL, (b + 1) * TL)
                        nc.vector.tensor_scalar(
                            h[:, c, sl], t1[:, sl],
                            w_cols[:, c:c + 1], b_cols[:, c:c + 1],
                            OP.mult, OP.add,
                            accum_out=accum_cols[b][:, c:c + 1])
            return h

        # ---- input embedding LN -> x ----
        xemb = act1.tile([128, DC, NT], F32, tag="kv")   # reuse kv slot
        nc.sync.dma_start(xemb[:], xembT.rearrange("(o p) f -> p o f", p=128))
        layer_norm(xemb, lncol("in_w"), lncol("in_b"), F32, out_tile=x)

        def dump(t):
            if xdbg_o is not None:
                nc.sync.dma_start(xdbg_o[:], t[:])

        # ---- deferred vocab-sharded heads: LN+AG now, matmuls drip-fed ----
        pending = []
        heg_sb_cache = {}

        def head_pre(j):
            if j < 2:
                wc, bc = lncol(f"ex_w_{j}"), lncol(f"ex_b_{j}")
                hm = [smp.tile([128, DC], F32, tag=f"hm{j}{b}", name="hm")
                      for b in range(B)]
                for b in range(B):
                    nc.vector.memset(hm[b][:], 0.0)
                he = layer_norm(x, wc, bc, BF16, accum_cols=hm)
                for b in range(B):
                    nc.sync.dma_start(hemean_o[:, j * 2 + b, :], hm[b][:])
            else:
                he = layer_norm(x, lncol("out_w"), lncol("out_b"), BF16)
            nc.sync.dma_start(heb[j].rearrange("p (o f) -> p o f", o=DC), he[:])
            nc.gpsimd.collective_compute(
                "AllGather", OP.bypass, replica_groups=RG,
                ins=[heb[j].ap().opt()], outs=[heg[j].ap().opt()])

        def queue_head_blocks(j):
            for vc in range(NV):
                def block(j=j, vc=vc):
                    if j not in heg_sb_cache:
                        hs = hegp.tile([128, R, DC, NT], BF16, tag="heg",
                                       name="heg_sb")
                        nc.sync.dma_start(
                            hs[:], heg[j].rearrange("(r p) (o f) -> p r o f",
                                                    p=128, o=DC))
                        heg_sb_cache.clear()
                        heg_sb_cache[j] = hs
                    hs = heg_sb_cache[j]
                    wh = wheadp.tile([128, DC, VC], BF16, tag="wh", name="wh")
                    nc.sync.dma_start(wh[:], headT[j][vc])
                    for mt in range(2 * R):
                        rb, jh = mt // 2, mt % 2
                        lps = psa.tile([128, VC], F32, tag="pa", name="lps")
                        for d in range(DC):
                            nc.tensor.matmul(
                                lps[:], hs[:, rb, d, jh * 128:(jh + 1) * 128],
                                wh[:, d, :], start=(d == 0), stop=(d == DC - 1))
                        ls = scr.tile([128, VC], F32, tag="lsb", name="ls")
                        nc.scalar.copy(ls[:], lps[:])
                        nc.sync.dma_start(
                            out_logits[j][mt * 128:(mt + 1) * 128,
                                          vc * VC:(vc + 1) * VC], ls[:])
                pending.append(block)

        reserve = [0]

        def drain_blocks(k):
            n = 0
            while len(pending) > reserve[0] and n < k:
                pending.pop(0)()
                n += 1

        # =================================================================
        for i in range(n_layers):
            # ---------------- TimeMix ----------------
            h = layer_norm(x, lncol(f"ln1_w_{i}"), lncol(f"ln1_b_{i}"), BF16)
            if stage == 0:
                hh = act1.tile([128, DC, NT], F32, tag="kv")
                for c in range(DC):
                    nc.vector.tensor_copy(hh[:, c, :], h[:, c, :])
                dump(hh)
                break

            # decay tables; w = min(exp(-t*lndec), 1e10), b = min(scale*1e10, 1)
            dec = smp.tile([128, DC], F32, tag="dec")
            lndec = smp.tile([128, DC], F32, tag="lndec")
            nc.scalar.activation(dec[:], decayT[:, :, i], AF.Sigmoid)
            nc.vector.tensor_scalar_max(dec[:], dec[:], 1e-7)
            nc.scalar.activation(lndec[:], dec[:], AF.Ln)
            scale_t = act1.tile([128, DC, TL], F32, tag="scale")
            for c in range(DC):
                sc0 = scr.tile([128, TL], F32, tag="sc0")
                nc.vector.tensor_scalar_mul(sc0[:], iota_b[:], lndec[:, c:c + 1])
                nc.scalar.activation(scale_t[:, c, :], sc0[:], AF.Exp)

            # k,v projections + kv product + carry sums
            kv = act1.tile([128, DC, NT], F32, tag="kv")
            r = act1.tile([128, DC, NT], F32, tag="r")
            ysum = smp.tile([128, 2 * DC], F32, tag="ysum")
            wkq = wvq = wrq = None
            for c in range(DC):
                if c % 2 == 0:
                    wkq = wddp.tile([128, 2, DC, 128], BF16, tag="wk", name="wk")
                    wvq = wddp.tile([128, 2, DC, 128], BF16, tag="wv", name="wv")
                    nc.sync.dma_start(wkq[:], w_qkv[i][1][c // 2 * 2:c // 2 * 2 + 2]
                                      .rearrange("q p o f -> p q o f"))
                    nc.sync.dma_start(wvq[:], w_qkv[i][2][c // 2 * 2:c // 2 * 2 + 2]
                                      .rearrange("q p o f -> p q o f"))
                ci = c % 2
                kps = psa.tile([128, NT], F32, tag="pa")
                vps = psb.tile([128, NT], F32, tag="pb")
                for d in range(DC):
                    nc.tensor.matmul(kps[:], wkq[:, ci, d, :], h[:, d, :],
                                     start=(d == 0), stop=(d == DC - 1))
                for d in range(DC):
                    nc.tensor.matmul(vps[:], wvq[:, ci, d, :], h[:, d, :],
                                     start=(d == 0), stop=(d == DC - 1))
                ksb = scr.tile([128, NT], F32, tag="ksb")
                nc.scalar.copy(ksb[:], kps[:])
                nc.vector.tensor_tensor(kv[:, c, :], ksb[:], vps[:], OP.mult)
                wtab = scr.tile([128, TL], F32, tag="wtab")
                nc.vector.tensor_scalar(wtab[:], iota_b[:], lndec[:, c:c + 1],
                                        -1.0, OP.mult, OP.mult)
                nc.scalar.activation(wtab[:], wtab[:], AF.Exp)
                nc.vector.tensor_scalar_min(wtab[:], wtab[:], 1e10)
                for b in range(B):
                    sc = scr.tile([128, TL], F32, tag="ysc")
                    nc.vector.tensor_tensor(sc[:], kv[:, c, b * TL:(b + 1) * TL],
                                            wtab[:], OP.mult)
                    nc.vector.tensor_reduce(
                        ysum[:, c * 2 + b:c * 2 + b + 1], sc[:],
                        axis=mybir.AxisListType.X, op=OP.add)
            if stage == 1:
                dump(kv)
                break
            nc.sync.dma_start(cc_carry_in[i][:], ysum[:])
            nc.gpsimd.collective_compute(
                "AllGather", OP.bypass, replica_groups=RG,
                ins=[cc_carry_in[i].ap().opt()], outs=[cc_carry_out[i].ap().opt()])

            for c in range(DC):
                if c % 2 == 0:
                    wrq = wddp.tile([128, 2, DC, 128], BF16, tag="wr", name="wr")
                    nc.sync.dma_start(wrq[:], w_qkv[i][0][c // 2 * 2:c // 2 * 2 + 2]
                                      .rearrange("q p o f -> p q o f"))
                rps = psa.tile([128, NT], F32, tag="pa")
                for d in range(DC):
                    nc.tensor.matmul(rps[:], wrq[:, c % 2, d, :], h[:, d, :],
                                     start=(d == 0), stop=(d == DC - 1))
                nc.scalar.activation(r[:, c, :], rps[:], AF.Sigmoid)
            if stage > 90:
                drain_blocks(1)

            # local scan (z written in place over kv)
            z = kv
            for c in range(DC):
                btab = scr.tile([128, TL], F32, tag="btab")
                nc.vector.tensor_scalar(btab[:], scale_t[:, c, :], 1e10, 1.0,
                                        OP.mult, OP.min)
                for b in range(B):
                    sl = slice(b * TL, (b + 1) * TL)
                    bkv = scr.tile([128, TL], F32, tag="bkv")
                    nc.vector.tensor_tensor(bkv[:], kv[:, c, sl], btab[:],
                                            OP.mult)
                    nc.vector.tensor_tensor_scan(
                        z[:, c, sl], dec[:, c:c + 1].to_broadcast((128, TL)),
                        bkv[:], 0.0, OP.mult, OP.add)
            if stage == 2:
                dump(z)
                break

            # carry prefix
            carry_all = smp.tile([128, R, 2 * DC], F32, tag="carry_all")
            nc.sync.dma_start(carry_all[:],
                              cc_carry_out[i].rearrange("(r p) f -> p r f", p=128))
            cmask = smp.tile([128, R, 2 * DC], F32, tag="cmask")
            nc.vector.tensor_tensor(
                cmask[:], carry_all[:],
                rmask_b[:, :, None].to_broadcast((128, R, 2 * DC)), OP.mult)
            cprev = smp.tile([128, 2 * DC], F32, tag="cprev")
            nc.vector.tensor_reduce(cprev[:],
                                    cmask[:].rearrange("p r f -> p f r"),
                                    axis=mybir.AxisListType.X, op=OP.add)

            # out = r * (scale*cprev + z); stats + bf16 cast (all DVE)
            out_bf = act1.tile([128, DC, NT], BF16, tag="out_bf")
            ssum = smp.tile([128, 2 * DC], F32, tag="ssum")
            ssq = smp.tile([128, 2 * DC], F32, tag="ssq")
            nc.vector.memset(ssum[:], 0.0)
            nc.vector.memset(ssq[:], 0.0)
            for c in range(DC):
                of = scr.tile([128, NT], F32, tag="outf")
                for b in range(B):
                    sl = slice(b * TL, (b + 1) * TL)
                    j = c * 2 + b
                    nc.vector.scalar_tensor_tensor(
                        of[:, sl], scale_t[:, c, :], cprev[:, j:j + 1],
                        z[:, c, sl], OP.mult, OP.add)
                    nc.vector.tensor_tensor(of[:, sl], r[:, c, sl], of[:, sl],
                                            OP.mult)
                    nc.scalar.activation(out_bf[:, c, sl], of[:, sl], AF.Copy,
                                         accum_out=ssum[:, j:j + 1])
                    sq2 = scr.tile([128, TL], F32, tag="gnsq")
                    nc.vector.scalar_tensor_tensor(
                        sq2[:], of[:, sl], 1.0, of[:, sl], OP.mult, OP.mult,
                        accum_out=ssq[:, j:j + 1])
            if stage == 3:
                oo = act1.tile([128, DC, NT], F32, tag="kv")
                for c in range(DC):
                    nc.vector.tensor_copy(oo[:, c, :], out_bf[:, c, :])
                dump(oo)
                break

            # GN stats partition-reduce + AllReduce
            sgn_ps = psa.tile([1, NT], F32, tag="pa", name="sgn_ps")
            sqn_ps = psb.tile([1, NT], F32, tag="pb", name="sqn_ps")
            nc.tensor.matmul(sgn_ps[:, :2 * DC], ones_col[:], ssum[:])
            nc.tensor.matmul(sqn_ps[:, :2 * DC], ones_col[:], ssq[:])
            st_row = smp.tile([1, 4], F32, tag="gnrow")
            nc.vector.tensor_reduce(
                st_row[:, 0:2],
                sgn_ps[:, :2 * DC].rearrange("p (c b) -> p b c", b=B),
                axis=mybir.AxisListType.X, op=OP.add)
            nc.vector.tensor_reduce(
                st_row[:, 2:4],
                sqn_ps[:, :2 * DC].rearrange("p (c b) -> p b c", b=B),
                axis=mybir.AxisListType.X, op=OP.add)
            nc.sync.dma_start(cc_gn_in[i][:], st_row[:])
            nc.gpsimd.collective_compute(
                "AllReduce", OP.add, replica_groups=RG,
                ins=[cc_gn_in[i].ap().opt()], outs=[cc_gn_out[i].ap().opt()])

            # finish GN stats: istd / -m*istd per batch
            g_st = smp.tile([1, 4], F32, tag="gst")
            nc.sync.dma_start(g_st[:], cc_gn_out[i][:])
            inv = 1.0 / (T * D)
            mrow = smp.tile([1, 2], F32, tag="gm")
            vrow = smp.tile([1, 2], F32, tag="gv")
            t2 = smp.tile([1, 2], F32, tag="gt2")
            nc.vector.tensor_scalar_mul(mrow[:], g_st[:, 0:2], inv)
            nc.vector.tensor_scalar_mul(vrow[:], g_st[:, 2:4], inv)
            nc.vector.tensor_tensor(t2[:], mrow[:], mrow[:], OP.mult)
            nc.vector.tensor_tensor(vrow[:], vrow[:], t2[:], OP.subtract)
            nc.vector.tensor_scalar_add(vrow[:], vrow[:], 1e-5)
            nc.scalar.sqrt(vrow[:], vrow[:])
            nc.vector.reciprocal(vrow[:], vrow[:])           # istd [1,2]
            nc.vector.tensor_tensor(mrow[:], mrow[:], vrow[:], OP.mult)
            nc.vector.tensor_scalar_mul(mrow[:], mrow[:], -1.0)  # -m*istd
            row4 = smp.tile([1, 4], F32, tag="grow4")
            nc.vector.tensor_copy(row4[:, 0:2], vrow[:])
            nc.vector.tensor_copy(row4[:, 2:4], mrow[:])
            bc4 = smp.tile([128, 4], F32, tag="gbc4")
            nc.gpsimd.partition_broadcast(bc4[:], row4[:])
            cvec = smp.tile([128, DC, B], F32, tag="cvec")
            for b in range(B):
                nc.vector.tensor_scalar_mul(cvec[:, :, b], gnaux[:, :, 2 * i],
                                            bc4[:, 2 + b:3 + b])
                nc.vector.tensor_tensor(cvec[:, :, b], cvec[:, :, b],
                                        gnaux[:, :, 2 * i + 1], OP.add)

            # P = out_bf @ WoT (gn_w folded); fixup; residual add
            woq = None
            for d in range(DC):
                if d % 2 == 0:
                    woq = wddp.tile([128, 2, DC, 128], BF16, tag="wo", name="wo")
                    nc.sync.dma_start(woq[:], w_qkv[i][3][d // 2 * 2:d // 2 * 2 + 2]
                                      .rearrange("q p o f -> p q o f"))
                P = psd.tile([128, NT], F32, tag="pd")
                for c in range(DC):
                    nc.tensor.matmul(P[:], woq[:, d % 2, c, :], out_bf[:, c, :],
                                     start=(c == 0), stop=(c == DC - 1))
                for b in range(B):
                    sl = slice(b * TL, (b + 1) * TL)
                    nc.vector.tensor_scalar(P[:, sl], P[:, sl],
                                            bc4[:, b:b + 1], cvec[:, d, b:b + 1],
                                            OP.mult, OP.add)
                nc.vector.tensor_tensor(x[:, d, :], x[:, d, :], P[:], OP.add)
            if stage > 90:
                drain_blocks(1)
            if stage == 4:
                dump(x)
                break

            # ---------------- ChannelMix ----------------
            h2 = layer_norm(x, lncol(f"ln2_w_{i}"), lncol(f"ln2_b_{i}"), BF16)
            for gh in range(2):                       # ffn halves
                g = act1.tile([128, FC // 2, NT], BF16, tag="g", name="g")
                for fg in range(4):                   # groups of 4 f-tiles
                    ft0 = gh * (FC // 2) + fg * 4
                    w1c = wddp.tile([128, 4, DC, 128], BF16, tag="w1", name="w1")
                    w2c = wddp.tile([128, 4, DC, 128], BF16, tag="w2", name="w2")
                    nc.sync.dma_start(w1c[:], w_12[i][0][ft0:ft0 + 4]
                                      .rearrange("q p o f -> p q o f"))
                    nc.sync.dma_start(w2c[:], w_12[i][1][ft0:ft0 + 4]
                                      .rearrange("q p o f -> p q o f"))
                    for fi in range(4):
                        aps_ = psa.tile([128, NT], F32, tag="pa")
                        bps_ = psb.tile([128, NT], F32, tag="pb")
                        for d in range(DC):
                            nc.tensor.matmul(aps_[:], w1c[:, fi, d, :],
                                             h2[:, d, :],
                                             start=(d == 0), stop=(d == DC - 1))
                        for d in range(DC):
                            nc.tensor.matmul(bps_[:], w2c[:, fi, d, :],
                                             h2[:, d, :],
                                             start=(d == 0), stop=(d == DC - 1))
                        sa = scr.tile([128, NT], F32, tag="silu")
                        nc.scalar.activation(sa[:], aps_[:], AF.Silu)
                        nc.vector.tensor_tensor(g[:, fg * 4 + fi, :], sa[:],
                                                bps_[:], OP.mult)
                for d in range(DC):
                    wcq = wcmp.tile([128, FC // 2, 128], BF16, tag="wcq")
                    nc.sync.dma_start(
                        wcq[:], w_cm[i][d, :, gh * (FC // 2):(gh + 1) * (FC // 2), :])
                    d2 = psd.tile([128, NT], F32, tag="pd")
                    for fh in range(FC // 2):
                        nc.tensor.matmul(d2[:], wcq[:, fh, :], g[:, fh, :],
                                         start=(fh == 0), stop=(fh == FC // 2 - 1))
                    nc.vector.tensor_tensor(x[:, d, :], x[:, d, :], d2[:], OP.add)

            # ---------------- early exits ----------------
            if do_heads and (i + 1) in EXITS:
                j = EXITS.index(i + 1)
                head_pre(j)
                queue_head_blocks(j)
                reserve[0] = 2

        # ---------------- final head ----------------
        if do_heads:
            head_pre(2)
            reserve[0] = 0
            queue_head_blocks(2)
            drain_blocks(len(pending))
        if xdbg_o is not None and stage > 90:
            nc.sync.dma_start(xdbg_o[:], x[:])

    nc.compile()
    return nc


# =====================================================================
# host side
# =====================================================================
_CACHE = {}


def _get_nc():
    if "nc" not in _CACHE:
        _CACHE["nc"] = _build()
    return _CACHE["nc"]


def _bf(a):
    return np.ascontiguousarray(a.astype(ml_dtypes.bfloat16))


def _f32(a):
    return np.ascontiguousarray(np.asarray(a, dtype=np.float32))


def _featmaj(vec):
    """[D] -> [128, DC] feature-major column block."""
    return np.ascontiguousarray(vec.reshape(DC, 128).T)


def _repack(Wout_in):
    """W [n_out, n_in] -> [out_hi, in_lo(128), in_hi, out_lo(128)] bf16."""
    no, ni = Wout_in.shape
    a = Wout_in.reshape(no // 128, 128, ni // 128, 128)
    return _bf(a.transpose(0, 3, 2, 1))


def _repack_cm(Wdf):
    """Wo_cm [D, DFF] -> [d_hi, f_lo(128), f_hi, d_lo(128)] bf16."""
    a = Wdf.reshape(DC, 128, FC, 128)
    return _bf(a.transpose(0, 3, 2, 1))


def _repack_head(Wvd):
    """W [VS, D] -> [NV, d_lo(128), DC, VC] bf16."""
    a = Wvd.reshape(NV, VC, DC, 128)
    return _bf(a.transpose(0, 3, 2, 1))


def kernel(idx, targets, embed, ln_in_w, ln_in_b, Wr, Wk, Wv, Wo_tm, decay,
           gn_w, gn_b, ln1_w, ln1_b, ln2_w, ln2_b, W1, W2, Wo_cm,
           ln_out_w, ln_out_b, exit_ln_w, exit_ln_b, exit_head,
           gate_w1, gate_b1, gate_w2, gate_b2):
    idx = np.asarray(idx)
    targets = np.asarray(targets)
    embed = _f32(embed)
    nc = _get_nc()

    # ---- shared (rank-independent) prep ----
    lnp = np.zeros((128, DC, NLN), np.float32)

    def setln(name, vec):
        lnp[:, :, _LN_COLS[name]] = _featmaj(_f32(vec))

    setln("in_w", ln_in_w); setln("in_b", ln_in_b)
    for i in range(L):
        setln(f"ln1_w_{i}", ln1_w[i]); setln(f"ln1_b_{i}", ln1_b[i])
        setln(f"ln2_w_{i}", ln2_w[i]); setln(f"ln2_b_{i}", ln2_b[i])
    setln("out_w", ln_out_w); setln("out_b", ln_out_b)
    for j in range(2):
        setln(f"ex_w_{j}", exit_ln_w[j]); setln(f"ex_b_{j}", exit_ln_b[j])

    decay = _f32(decay)
    decayT = np.ascontiguousarray(
        np.stack([_featmaj(decay[i]) for i in range(L)], axis=-1))

    gnaux = np.zeros((128, DC, 2 * L), np.float32)
    Wo_tm = _f32(Wo_tm); gn_w = _f32(gn_w); gn_b = _f32(gn_b)
    for i in range(L):
        gnaux[:, :, 2 * i] = _featmaj(Wo_tm[i] @ gn_w[i])
        gnaux[:, :, 2 * i + 1] = _featmaj(Wo_tm[i] @ gn_b[i])

    shared = {"lnp": lnp, "decayT": decayT, "gnaux": gnaux}
    for i in range(L):
        shared[f"wr{i}"] = _repack(_f32(Wr[i]))
        shared[f"wk{i}"] = _repack(_f32(Wk[i]))
        shared[f"wv{i}"] = _repack(_f32(Wv[i]))
        shared[f"wo{i}"] = _repack(Wo_tm[i] * gn_w[i][None, :])
        shared[f"wf1{i}"] = _repack(_f32(W1[i]))
        shared[f"wf2{i}"] = _repack(_f32(W2[i]))
        shared[f"wcm{i}"] = _repack_cm(_f32(Wo_cm[i]))

    x_emb = embed[np.asarray(idx, dtype=np.int64)]        # [B, T, D]
    exit_head = _f32(exit_head)

    in_maps = []
    for r in range(R):
        m = dict(shared)
        sl = x_emb[:, r * TL:(r + 1) * TL, :]              # [B, TL, D]
        m["xembT"] = np.ascontiguousarray(
            sl.transpose(2, 0, 1).reshape(D, NT).astype(np.float32))
        m["iota"] = (np.arange(TL, dtype=np.float32) + r * TL).reshape(1, TL)
        m["rmask"] = (np.arange(R) < r).astype(np.float32).reshape(1, R)
        vs = slice(r * VS, (r + 1) * VS)
        m["headT0"] = _repack_head(exit_head[0, vs, :])
        m["headT1"] = _repack_head(exit_head[1, vs, :])
        m["headT2"] = _repack_head(embed[vs, :])
        in_maps.append(m)

    _CACHE["in_maps"] = in_maps
    res = run_bass_kernel_spmd(nc, in_maps, core_ids=list(range(R)))

    # ---- unshard ----
    def assemble(name):
        shards = []
        for r in range(R):
            a = res.results[r][name]                       # [2048, VS]
            a = a.reshape(R, B, TL, VS).transpose(1, 0, 2, 3).reshape(B, T, VS)
            shards.append(a)
        return np.concatenate(shards, axis=-1)             # [B, T, V]

    final_logits = assemble("logits")
    e_logits = [assemble("exit0"), assemble("exit1")]

    hm = np.zeros((2, B, D), np.float64)
    for r in range(R):
        a = res.results[r]["hemean"]                        # [128, 4, DC]
        for j in range(2):
            for b in range(B):
                hm[j, b] += a[:, j * 2 + b, :].T.reshape(D).astype(np.float64)
    hm /= T

    # ---- loss in f64 ----
    tgt = np.asarray(targets, dtype=np.int64)

    def ce_and_stats(lg):
        lg = lg.astype(np.float64)
        mx = lg.max(-1, keepdims=True)
        ex = np.exp(lg - mx)
        Z = ex.sum(-1, keepdims=True)
        lse = (mx + np.log(Z))[..., 0]                     # [B,T]
        tl_ = np.take_along_axis(lg, tgt[..., None], -1)[..., 0]
        ce = float((lse - tl_).mean())
        pred = lg.argmax(-1)
        p = ex / Z
        ent = lse - (p * lg).sum(-1)                       # [B,T]
        return ce, pred, ent

    ce_f, pred_f, _ = ce_and_stats(final_logits)
    loss = WEIGHTS[-1] * ce_f
    max_ent = np.log(V)
    gate_w1 = _f32(gate_w1); gate_b1 = _f32(gate_b1)
    gate_w2 = _f32(gate_w2); gate_b2 = _f32(gate_b2)
    for j in range(2):
        ce_j, pred_j, ent_j = ce_and_stats(e_logits[j])
        loss += WEIGHTS[j] * ce_j
        agree = (pred_j == pred_f).astype(np.float64).mean(-1, keepdims=True)
        gact = np.maximum(hm[j] @ gate_w1[j].T.astype(np.float64)
                          + gate_b1[j].astype(np.float64), 0.0)
        conf = 1.0 / (1.0 + np.exp(-(gact @ gate_w2[j].T.astype(np.float64)
                                     + gate_b2[j].astype(np.float64))))
        c = np.clip(conf, 1e-7, 1.0 - 1e-7)
        loss += 0.5 * float(-(agree * np.log(c)
                              + (1.0 - agree) * np.log(1.0 - c)).mean())
        overconf = (1.0 - ent_j / max_ent) * (pred_j != pred_f)
        loss += 0.1 * float(overconf.mean())

    return final_logits.astype(np.float32), np.float32(loss)


# revision 29
# speedup vs baseline: 1.1133x; 1.0000x over previous
"""Trainium2 Bass kernel for nn_AdaptiveDepthRWKV (8 NeuronCores, SPMD).

Sharding: token-parallel backbone (core r owns timesteps [128r,128(r+1)) of both
batches; activations feature-major [d, tok] in SBUF), weights replicated and
streamed bf16 from HBM with per-partition-contiguous layouts. The RWKV
decay-cumsum runs as a native DVE tensor_tensor_scan recurrence
z_t = dec*z_{t-1} + b_t*kv_t with a tiny carry all-gather across cores.
GroupNorm uses a linearity trick: the Wo_tm matmul runs on un-normalized data,
then an istd*P + cvec rank-1 fixup lands after the stats all-reduce. Heads are
vocab-sharded (4000 cols/core) after all-gathering the LN'd hidden state. Loss
terms are assembled on host in f64 from the full logits.
"""
import sys

sys.path.insert(0, "/opt/trn_rl_repo")

import numpy as np
import ml_dtypes

import concourse.bass as bass
import concourse.mybir as mybir
import concourse.tile as tile
from concourse import bacc
from concourse.bass_utils import run_bass_kernel_spmd

F32 = mybir.dt.float32
BF16 = mybir.dt.bfloat16
AF = mybir.ActivationFunctionType
OP = mybir.AluOpType

V, D, DFF, L, B, T = 32000, 1024, 4096, 12, 2, 1024
R = 8                 # cores
TL = T // R           # 128 timesteps per core
NT = B * TL           # 256 local tokens, free order: [b0 t0..127, b1 t0..127]
VS = V // R           # 4000 vocab cols per core
DC = D // 128         # 8 hidden chunks
FC = DFF // 128       # 32 ffn chunks
NV = 8                # vocab chunks per core
VC = VS // NV         # 500
EXITS = (4, 8)
WEIGHTS = (0.3, 0.5, 1.0)

_LN_COLS = {}


def _lncol(name):
    _LN_COLS[name] = len(_LN_COLS)


_lncol("in_w"); _lncol("in_b")
for _i in range(L):
    for _n in ("ln1_w", "ln1_b", "ln2_w", "ln2_b"):
        _lncol(f"{_n}_{_i}")
_lncol("out_w"); _lncol("out_b")
for _j in range(2):
    _lncol(f"ex_w_{_j}"); _lncol(f"ex_b_{_j}")
NLN = len(_LN_COLS)


def _build(n_layers=L, do_heads=True, dbg_x=False, stage=99):
    nc = bacc.Bacc("TRN2", target_bir_lowering=False, debug=False, num_devices=R)

    # ---------------- inputs ----------------
    xembT = nc.dram_tensor("xembT", [D, NT], F32, kind="ExternalInput")
    lnp_d = nc.dram_tensor("lnp", [128, DC, NLN], F32, kind="ExternalInput")
    decayT_d = nc.dram_tensor("decayT", [128, DC, L], F32, kind="ExternalInput")
    gnaux_d = nc.dram_tensor("gnaux", [128, DC, 2 * L], F32, kind="ExternalInput")
    iota_d = nc.dram_tensor("iota", [1, TL], F32, kind="ExternalInput")
    rmask_d = nc.dram_tensor("rmask", [1, R], F32, kind="ExternalInput")
    # weights repacked [out_hi, in_lo(128), in_hi, out_lo] (partition-contiguous)
    w_qkv = [[nc.dram_tensor(f"w{n}{i}", [DC, 128, DC, 128], BF16,
                             kind="ExternalInput")
              for n in ("r", "k", "v", "o")] for i in range(n_layers)]
    w_12 = [[nc.dram_tensor(f"wf{n}{i}", [FC, 128, DC, 128], BF16,
                            kind="ExternalInput")
             for n in (1, 2)] for i in range(n_layers)]
    w_cm = [nc.dram_tensor(f"wcm{i}", [DC, 128, FC, 128], BF16,
                           kind="ExternalInput") for i in range(n_layers)]
    headT = [nc.dram_tensor(f"headT{j}", [NV, 128, DC, VC], BF16,
                            kind="ExternalInput")
             for j in range(3 if do_heads else 0)]

    # ---------------- outputs ----------------
    out_logits = [nc.dram_tensor(n, [B * T, VS], F32, kind="ExternalOutput")
                  for n in ("exit0", "exit1", "logits")]
    hemean_o = nc.dram_tensor("hemean", [128, 4, DC], F32, kind="ExternalOutput")
    xdbg_o = (nc.dram_tensor("xdbg", [128, DC, NT], F32, kind="ExternalOutput")
              if dbg_x else None)

    # ---------------- collective buffers ----------------
    cc_carry_in = [nc.dram_tensor(f"cyi{i}", [128, 2 * DC], F32) for i in range(L)]
    cc_carry_out = [nc.dram_tensor(f"cyo{i}", [128 * R, 2 * DC], F32,
                                   addr_space="Shared") for i in range(L)]
    cc_gn_in = [nc.dram_tensor(f"gni{i}", [1, 4], F32) for i in range(L)]
    cc_gn_out = [nc.dram_tensor(f"gno{i}", [1, 4], F32, addr_space="Shared")
                 for i in range(L)]
    heb = [nc.dram_tensor(f"heb{j}", [128, DC * NT], BF16) for j in range(3)]
    heg = [nc.dram_tensor(f"heg{j}", [128 * R, DC * NT], BF16,
                          addr_space="Shared") for j in range(3)]
    RG = [list(range(R))]

    with tile.TileContext(nc) as tc, \
         tc.tile_pool(name="persist", bufs=1) as pers, \
         tc.tile_pool(name="act1", bufs=1) as act1, \
         tc.tile_pool(name="act2", bufs=2) as act2, \
         tc.tile_pool(name="scratch", bufs=2) as scr, \
         tc.tile_pool(name="small", bufs=2) as smp, \
         tc.tile_pool(name="wdd", bufs=2) as wddp, \
         tc.tile_pool(name="wcmp", bufs=2) as wcmp, \
         tc.tile_pool(name="whead", bufs=2) as wheadp, \
         tc.tile_pool(name="hegp", bufs=1) as hegp, \
         tc.tile_pool(name="psa", bufs=3, space="PSUM") as psa, \
         tc.tile_pool(name="psb", bufs=3, space="PSUM") as psb, \
         tc.tile_pool(name="psd", bufs=2, space="PSUM") as psd:

        # ---- persistent setup ----
        x = pers.tile([128, DC, NT], F32)          # residual master, feature-major
        lnp = pers.tile([128, DC, NLN], F32)
        decayT = pers.tile([128, DC, L], F32)
        gnaux = pers.tile([128, DC, 2 * L], F32)
        iota_b = pers.tile([128, TL], F32)
        rmask_b = pers.tile([128, R], F32)
        ones_col = pers.tile([128, 1], F32)
        nc.sync.dma_start(lnp[:], lnp_d[:])
        nc.sync.dma_start(decayT[:], decayT_d[:])
        nc.sync.dma_start(gnaux[:], gnaux_d[:])
        iota_r = pers.tile([1, TL], F32)
        rmask_r = pers.tile([1, R], F32)
        nc.sync.dma_start(iota_r[:], iota_d[:])
        nc.sync.dma_start(rmask_r[:], rmask_d[:])
        nc.gpsimd.partition_broadcast(iota_b[:], iota_r[:])
        nc.gpsimd.partition_broadcast(rmask_b[:], rmask_r[:])
        nc.vector.memset(ones_col[:], 1.0)

        def lncol(name):
            return lnp[:, :, _LN_COLS[name]]

        # ---- feature-major layernorm ----
        def layer_norm(src, w_cols, b_cols, out_dtype, accum_cols=None,
                       out_tile=None):
            """src: [128, DC, NT] f32. Returns h [128, DC, NT] out_dtype."""
            sum_ps = psa.tile([1, NT], F32, tag="pa", name="sum_ps")
            sq_ps = psb.tile([1, NT], F32, tag="pb", name="sq_ps")
            for c in range(DC):
                sq = scr.tile([128, NT], F32, tag="lnsqt")
                nc.scalar.activation(sq[:], src[:, c, :], AF.Square)
                nc.tensor.matmul(sum_ps[:], ones_col[:], src[:, c, :],
                                 start=(c == 0), stop=(c == DC - 1))
                nc.tensor.matmul(sq_ps[:], ones_col[:], sq[:],
                                 start=(c == 0), stop=(c == DC - 1))
            mrow = smp.tile([1, NT], F32, tag="lnm")
            var = smp.tile([1, NT], F32, tag="lnv")
            msq = smp.tile([1, NT], F32, tag="lnmsq")
            nc.vector.tensor_scalar_mul(mrow[:], sum_ps[:], 1.0 / D)
            nc.vector.tensor_scalar_mul(var[:], sq_ps[:], 1.0 / D)
            nc.vector.tensor_tensor(msq[:], mrow[:], mrow[:], OP.mult)
            nc.vector.tensor_tensor(var[:], var[:], msq[:], OP.subtract)
            nc.vector.tensor_scalar_add(var[:], var[:], 1e-5)
            nc.scalar.sqrt(var[:], var[:])
            nc.vector.reciprocal(var[:], var[:])          # istd row
            nc.vector.tensor_tensor(mrow[:], mrow[:], var[:], OP.mult)  # m*istd
            ib = smp.tile([128, NT], F32, tag="lnib")
            mib = smp.tile([128, NT], F32, tag="lnmib")
            nc.gpsimd.partition_broadcast(ib[:], var[:])
            nc.gpsimd.partition_broadcast(mib[:], mrow[:])
            if out_tile is None:
                h = act2.tile([128, DC, NT], out_dtype, tag="h")
            else:
                h = out_tile
            for c in range(DC):
                t1 = scr.tile([128, NT], F32, tag="lnt1")
                nc.vector.tensor_tensor(t1[:], src[:, c, :], ib[:], OP.mult)
                nc.vector.tensor_tensor(t1[:], t1[:], mib[:], OP.subtract)
                if accum_cols is None:
                    nc.scalar.activation(h[:, c, :], t1[:], AF.Identity,
                                         bias=b_cols[:, c:c + 1],
                                         scale=w_cols[:, c:c + 1])
                else:
                    for b in range(B):
                        sl = slice(b * TL, (b + 1) * TL)
                        nc.scalar.activation(h[:, c, sl], t1[:, sl], AF.Identity,
                                             bias=b_cols[:, c:c + 1],
                                             scale=w_cols[:, c:c + 1],
                                             accum_out=accum_cols[b][:, c:c + 1])
            return h

        # ---- input embedding LN -> x ----
        xemb = act1.tile([128, DC, NT], F32, tag="kv")   # reuse kv slot
        nc.sync.dma_start(xemb[:], xembT.rearrange("(o p) f -> p o f", p=128))
        layer_norm(xemb, lncol("in_w"), lncol("in_b"), F32, out_tile=x)

        def dump(t):
            if xdbg_o is not None:
                nc.sync.dma_start(xdbg_o[:], t[:])

        # ---- deferred vocab-sharded heads: LN+AG now, matmuls drip-fed ----
        pending = []
        heg_sb_cache = {}

        def head_pre(j):
            if j < 2:
                wc, bc = lncol(f"ex_w_{j}"), lncol(f"ex_b_{j}")
                hm = [smp.tile([128, DC], F32, tag=f"hm{j}{b}", name="hm")
                      for b in range(B)]
                for b in range(B):
                    nc.vector.memset(hm[b][:], 0.0)
                he = layer_norm(x, wc, bc, BF16, accum_cols=hm)
                for b in range(B):
                    nc.sync.dma_start(hemean_o[:, j * 2 + b, :], hm[b][:])
            else:
                he = layer_norm(x, lncol("out_w"), lncol("out_b"), BF16)
            nc.sync.dma_start(heb[j].rearrange("p (o f) -> p o f", o=DC), he[:])
            nc.gpsimd.collective_compute(
                "AllGather", OP.bypass, replica_groups=RG,
                ins=[heb[j].ap().opt()], outs=[heg[j].ap().opt()])

        wh_cache = {}

        def queue_head_blocks(j):
            for vc in range(NV):
                for mt0 in range(0, 2 * R, 4):
                    def block(j=j, vc=vc, mt0=mt0):
                        if j not in heg_sb_cache:
                            hs = hegp.tile([128, R, DC, NT], BF16, tag="heg",
                                           name="heg_sb")
                            nc.sync.dma_start(
                                hs[:], heg[j].rearrange(
                                    "(r p) (o f) -> p r o f", p=128, o=DC))
                            heg_sb_cache.clear()
                            heg_sb_cache[j] = hs
                        hs = heg_sb_cache[j]
                        if (j, vc) not in wh_cache:
                            wh = wheadp.tile([128, DC, VC], BF16, tag="wh",
                                             name="wh")
                            nc.sync.dma_start(wh[:], headT[j][vc])
                            wh_cache.clear()
                            wh_cache[(j, vc)] = wh
                        wh = wh_cache[(j, vc)]
                        for mt in range(mt0, mt0 + 4):
                            rb, jh = mt // 2, mt % 2
                            lps = psa.tile([128, VC], F32, tag="pa", name="lps")
                            for d in range(DC):
                                nc.tensor.matmul(
                                    lps[:], hs[:, rb, d, jh * 128:(jh + 1) * 128],
                                    wh[:, d, :], start=(d == 0),
                                    stop=(d == DC - 1))
                            ls = scr.tile([128, VC], F32, tag="lsb", name="ls")
                            nc.scalar.copy(ls[:], lps[:])
                            nc.sync.dma_start(
                                out_logits[j][mt * 128:(mt + 1) * 128,
                                              vc * VC:(vc + 1) * VC], ls[:])
                    pending.append(block)

        reserve = [0]

        def drain_blocks(k):
            n = 0
            while len(pending) > reserve[0] and n < k:
                pending.pop(0)()
                n += 1

        # =================================================================
        for i in range(n_layers):
            # ---------------- TimeMix ----------------
            h = layer_norm(x, lncol(f"ln1_w_{i}"), lncol(f"ln1_b_{i}"), BF16)
            if stage == 0:
                hh = act1.tile([128, DC, NT], F32, tag="kv")
                for c in range(DC):
                    nc.vector.tensor_copy(hh[:, c, :], h[:, c, :])
                dump(hh)
                break

            if stage > 90:
                drain_blocks(1)
            # decay tables; w = min(exp(-t*lndec), 1e10), b = min(scale*1e10, 1)
            dec = smp.tile([128, DC], F32, tag="dec")
            lndec = smp.tile([128, DC], F32, tag="lndec")
            nc.scalar.activation(dec[:], decayT[:, :, i], AF.Sigmoid)
            nc.vector.tensor_scalar_max(dec[:], dec[:], 1e-7)
            nc.scalar.activation(lndec[:], dec[:], AF.Ln)
            scale_t = act1.tile([128, DC, TL], F32, tag="scale")
            for c in range(DC):
                sc0 = scr.tile([128, TL], F32, tag="sc0")
                nc.vector.tensor_scalar_mul(sc0[:], iota_b[:], lndec[:, c:c + 1])
                nc.scalar.activation(scale_t[:, c, :], sc0[:], AF.Exp)

            # k,v projections + kv product + carry sums
            kv = act1.tile([128, DC, NT], F32, tag="kv")
            r = act1.tile([128, DC, NT], F32, tag="r")
            ysum = smp.tile([128, 2 * DC], F32, tag="ysum")
            wkq = wvq = wrq = None
            for c in range(DC):
                if c % 2 == 0:
                    wkq = wddp.tile([128, 2, DC, 128], BF16, tag="wk", name="wk")
                    wvq = wddp.tile([128, 2, DC, 128], BF16, tag="wv", name="wv")
                    nc.sync.dma_start(wkq[:], w_qkv[i][1][c // 2 * 2:c // 2 * 2 + 2]
                                      .rearrange("q p o f -> p q o f"))
                    nc.sync.dma_start(wvq[:], w_qkv[i][2][c // 2 * 2:c // 2 * 2 + 2]
                                      .rearrange("q p o f -> p q o f"))
                ci = c % 2
                kps = psa.tile([128, NT], F32, tag="pa")
                vps = psb.tile([128, NT], F32, tag="pb")
                for d in range(DC):
                    nc.tensor.matmul(kps[:], wkq[:, ci, d, :], h[:, d, :],
                                     start=(d == 0), stop=(d == DC - 1))
                for d in range(DC):
                    nc.tensor.matmul(vps[:], wvq[:, ci, d, :], h[:, d, :],
                                     start=(d == 0), stop=(d == DC - 1))
                ksb = scr.tile([128, NT], F32, tag="ksb")
                nc.scalar.copy(ksb[:], kps[:])
                nc.vector.tensor_tensor(kv[:, c, :], ksb[:], vps[:], OP.mult)
                wtab = scr.tile([128, TL], F32, tag="wtab")
                nc.vector.tensor_scalar(wtab[:], iota_b[:], lndec[:, c:c + 1],
                                        -1.0, OP.mult, OP.mult)
                nc.scalar.activation(wtab[:], wtab[:], AF.Exp)
                nc.vector.tensor_scalar_min(wtab[:], wtab[:], 1e10)
                for b in range(B):
                    sc = scr.tile([128, TL], F32, tag="ysc")
                    nc.vector.tensor_tensor(sc[:], kv[:, c, b * TL:(b + 1) * TL],
                                            wtab[:], OP.mult)
                    nc.vector.tensor_reduce(
                        ysum[:, c * 2 + b:c * 2 + b + 1], sc[:],
                        axis=mybir.AxisListType.X, op=OP.add)
            if stage == 1:
                dump(kv)
                break
            if stage > 90:
                drain_blocks(1)
            nc.sync.dma_start(cc_carry_in[i][:], ysum[:])
            nc.gpsimd.collective_compute(
                "AllGather", OP.bypass, replica_groups=RG,
                ins=[cc_carry_in[i].ap().opt()], outs=[cc_carry_out[i].ap().opt()])

            for c in range(DC):
                if c % 2 == 0:
                    wrq = wddp.tile([128, 2, DC, 128], BF16, tag="wr", name="wr")
                    nc.sync.dma_start(wrq[:], w_qkv[i][0][c // 2 * 2:c // 2 * 2 + 2]
                                      .rearrange("q p o f -> p q o f"))
                rps = psa.tile([128, NT], F32, tag="pa")
                for d in range(DC):
                    nc.tensor.matmul(rps[:], wrq[:, c % 2, d, :], h[:, d, :],
                                     start=(d == 0), stop=(d == DC - 1))
                nc.scalar.activation(r[:, c, :], rps[:], AF.Sigmoid)
            if stage > 90:
                drain_blocks(1)

            # local scan (z written in place over kv)
            z = kv
            for c in range(DC):
                btab = scr.tile([128, TL], F32, tag="btab")
                nc.vector.tensor_scalar(btab[:], scale_t[:, c, :], 1e10, 1.0,
                                        OP.mult, OP.min)
                for b in range(B):
                    sl = slice(b * TL, (b + 1) * TL)
                    bkv = scr.tile([128, TL], F32, tag="bkv")
                    nc.vector.tensor_tensor(bkv[:], kv[:, c, sl], btab[:],
                                            OP.mult)
                    nc.vector.tensor_tensor_scan(
                        z[:, c, sl], dec[:, c:c + 1].to_broadcast((128, TL)),
                        bkv[:], 0.0, OP.mult, OP.add)
            if stage == 2:
                dump(z)
                break

            # carry prefix
            carry_all = smp.tile([128, R, 2 * DC], F32, tag="carry_all")
            nc.sync.dma_start(carry_all[:],
                              cc_carry_out[i].rearrange("(r p) f -> p r f", p=128))
            cmask = smp.tile([128, R, 2 * DC], F32, tag="cmask")
            nc.vector.tensor_tensor(
                cmask[:], carry_all[:],
                rmask_b[:, :, None].to_broadcast((128, R, 2 * DC)), OP.mult)
            cprev = smp.tile([128, 2 * DC], F32, tag="cprev")
            nc.vector.tensor_reduce(cprev[:],
                                    cmask[:].rearrange("p r f -> p f r"),
                                    axis=mybir.AxisListType.X, op=OP.add)

            # out = r * (scale*cprev + z); stats + bf16 cast (all DVE)
            out_bf = act1.tile([128, DC, NT], BF16, tag="out_bf")
            ssum = smp.tile([128, 2 * DC], F32, tag="ssum")
            ssq = smp.tile([128, 2 * DC], F32, tag="ssq")
            nc.vector.memset(ssum[:], 0.0)
            nc.vector.memset(ssq[:], 0.0)
            for c in range(DC):
                of = scr.tile([128, NT], F32, tag="outf")
                for b in range(B):
                    sl = slice(b * TL, (b + 1) * TL)
                    j = c * 2 + b
                    nc.vector.scalar_tensor_tensor(
                        of[:, sl], scale_t[:, c, :], cprev[:, j:j + 1],
                        z[:, c, sl], OP.mult, OP.add)
                    nc.vector.tensor_tensor(of[:, sl], r[:, c, sl], of[:, sl],
                                            OP.mult)
                    nc.scalar.activation(out_bf[:, c, sl], of[:, sl], AF.Copy,
                                         accum_out=ssum[:, j:j + 1])
                    sq2 = scr.tile([128, TL], F32, tag="gnsq")
                    nc.vector.scalar_tensor_tensor(
                        sq2[:], of[:, sl], 1.0, of[:, sl], OP.mult, OP.mult,
                        accum_out=ssq[:, j:j + 1])
            if stage == 3:
                oo = act1.tile([128, DC, NT], F32, tag="kv")
                for c in range(DC):
                    nc.vector.tensor_copy(oo[:, c, :], out_bf[:, c, :])
                dump(oo)
                break

            # GN stats partition-reduce + AllReduce
            sgn_ps = psa.tile([1, NT], F32, tag="pa", name="sgn_ps")
            sqn_ps = psb.tile([1, NT], F32, tag="pb", name="sqn_ps")
            nc.tensor.matmul(sgn_ps[:, :2 * DC], ones_col[:], ssum[:])
            nc.tensor.matmul(sqn_ps[:, :2 * DC], ones_col[:], ssq[:])
            st_row = smp.tile([1, 4], F32, tag="gnrow")
            nc.vector.tensor_reduce(
                st_row[:, 0:2],
                sgn_ps[:, :2 * DC].rearrange("p (c b) -> p b c", b=B),
                axis=mybir.AxisListType.X, op=OP.add)
            nc.vector.tensor_reduce(
                st_row[:, 2:4],
                sqn_ps[:, :2 * DC].rearrange("p (c b) -> p b c", b=B),
                axis=mybir.AxisListType.X, op=OP.add)
            nc.sync.dma_start(cc_gn_in[i][:], st_row[:])
            nc.gpsimd.collective_compute(
                "AllReduce", OP.add, replica_groups=RG,
                ins=[cc_gn_in[i].ap().opt()], outs=[cc_gn_out[i].ap().opt()])

            # finish GN stats: istd / -m*istd per batch
            g_st = smp.tile([1, 4], F32, tag="gst")
            nc.sync.dma_start(g_st[:], cc_gn_out[i][:])
            inv = 1.0 / (T * D)
            mrow = smp.tile([1, 2], F32, tag="gm")
            vrow = smp.tile([1, 2], F32, tag="gv")
            t2 = smp.tile([1, 2], F32, tag="gt2")
            nc.vector.tensor_scalar_mul(mrow[:], g_st[:, 0:2], inv)
            nc.vector.tensor_scalar_mul(vrow[:], g_st[:, 2:4], inv)
            nc.vector.tensor_tensor(t2[:], mrow[:], mrow[:], OP.mult)
            nc.vector.tensor_tensor(vrow[:], vrow[:], t2[:], OP.subtract)
            nc.vector.tensor_scalar_add(vrow[:], vrow[:], 1e-5)
            nc.scalar.sqrt(vrow[:], vrow[:])
            nc.vector.reciprocal(vrow[:], vrow[:])           # istd [1,2]
            nc.vector.tensor_tensor(mrow[:], mrow[:], vrow[:], OP.mult)
            nc.vector.tensor_scalar_mul(mrow[:], mrow[:], -1.0)  # -m*istd
            row4 = smp.tile([1, 4], F32, tag="grow4")
            nc.vector.tensor_copy(row4[:, 0:2], vrow[:])
            nc.vector.tensor_copy(row4[:, 2:4], mrow[:])
            bc4 = smp.tile([128, 4], F32, tag="gbc4")
            nc.gpsimd.partition_broadcast(bc4[:], row4[:])
            cvec = smp.tile([128, DC, B], F32, tag="cvec")
            for b in range(B):
                nc.vector.tensor_scalar_mul(cvec[:, :, b], gnaux[:, :, 2 * i],
                                            bc4[:, 2 + b:3 + b])
                nc.vector.tensor_tensor(cvec[:, :, b], cvec[:, :, b],
                                        gnaux[:, :, 2 * i + 1], OP.add)

            # P = out_bf @ WoT (gn_w folded); fixup; residual add
            woq = None
            for d in range(DC):
                if d % 2 == 0:
                    woq = wddp.tile([128, 2, DC, 128], BF16, tag="wo", name="wo")
                    nc.sync.dma_start(woq[:], w_qkv[i][3][d // 2 * 2:d // 2 * 2 + 2]
                                      .rearrange("q p o f -> p q o f"))
                P = psd.tile([128, NT], F32, tag="pd")
                for c in range(DC):
                    nc.tensor.matmul(P[:], woq[:, d % 2, c, :], out_bf[:, c, :],
                                     start=(c == 0), stop=(c == DC - 1))
                for b in range(B):
                    sl = slice(b * TL, (b + 1) * TL)
                    nc.vector.tensor_scalar(P[:, sl], P[:, sl],
                                            bc4[:, b:b + 1], cvec[:, d, b:b + 1],
                                            OP.mult, OP.add)
                nc.vector.tensor_tensor(x[:, d, :], x[:, d, :], P[:], OP.add)
            if stage > 90:
                drain_blocks(2)
            if stage == 4:
                dump(x)
                break

            # ---------------- ChannelMix ----------------
            h2 = layer_norm(x, lncol(f"ln2_w_{i}"), lncol(f"ln2_b_{i}"), BF16)
            for gh in range(2):                       # ffn halves
                g = act1.tile([128, FC // 2, NT], BF16, tag="g", name="g")
                for fg in range(4):                   # groups of 4 f-tiles
                    ft0 = gh * (FC // 2) + fg * 4
                    w1c = wddp.tile([128, 4, DC, 128], BF16, tag="w1", name="w1")
                    w2c = wddp.tile([128, 4, DC, 128], BF16, tag="w2", name="w2")
                    nc.sync.dma_start(w1c[:], w_12[i][0][ft0:ft0 + 4]
                                      .rearrange("q p o f -> p q o f"))
                    nc.sync.dma_start(w2c[:], w_12[i][1][ft0:ft0 + 4]
                                      .rearrange("q p o f -> p q o f"))
                    for fi in range(4):
                        aps_ = psa.tile([128, NT], F32, tag="pa")
                        bps_ = psb.tile([128, NT], F32, tag="pb")
                        for d in range(DC):
                            nc.tensor.matmul(aps_[:], w1c[:, fi, d, :],
                                             h2[:, d, :],
                                             start=(d == 0), stop=(d == DC - 1))
                        for d in range(DC):
                            nc.tensor.matmul(bps_[:], w2c[:, fi, d, :],
                                             h2[:, d, :],
                                             start=(d == 0), stop=(d == DC - 1))
                        sa = scr.tile([128, NT], F32, tag="silu")
                        nc.scalar.activation(sa[:], aps_[:], AF.Silu)
                        nc.vector.tensor_tensor(g[:, fg * 4 + fi, :], sa[:],
                                                bps_[:], OP.mult)
                for d in range(DC):
                    wcq = wcmp.tile([128, FC // 2, 128], BF16, tag="wcq")
                    nc.sync.dma_start(
                        wcq[:], w_cm[i][d, :, gh * (FC // 2):(gh + 1) * (FC // 2), :])
                    d2 = psd.tile([128, NT], F32, tag="pd")
                    for fh in range(FC // 2):
                        nc.tensor.matmul(d2[:], wcq[:, fh, :], g[:, fh, :],
                                         start=(fh == 0), stop=(fh == FC // 2 - 1))
                    nc.vector.tensor_tensor(x[:, d, :], x[:, d, :], d2[:], OP.add)
                if stage > 90:
                    drain_blocks(1)
            if stage > 90:
                drain_blocks(1)

            # ---------------- early exits ----------------
            if do_heads and (i + 1) in EXITS:
                j = EXITS.index(i + 1)
                drain_blocks(2)
                head_pre(j)
                queue_head_blocks(j)
                reserve[0] = 6

        # ---------------- final head ----------------
        if do_heads:
            head_pre(2)
            reserve[0] = 0
            queue_head_blocks(2)
            drain_blocks(len(pending))
        if xdbg_o is not None and stage > 90:
            nc.sync.dma_start(xdbg_o[:], x[:])

    nc.compile()
    return nc


# =====================================================================
# host side
# =====================================================================
_CACHE = {}


def _get_nc():
    if "nc" not in _CACHE:
        _CACHE["nc"] = _build()
    return _CACHE["nc"]


def _bf(a):
    return np.ascontiguousarray(a.astype(ml_dtypes.bfloat16))


def _f32(a):
    return np.ascontiguousarray(np.asarray(a, dtype=np.float32))


def _featmaj(vec):
    """[D] -> [128, DC] feature-major column block."""
    return np.ascontiguousarray(vec.reshape(DC, 128).T)


def _repack(Wout_in):
    """W [n_out, n_in] -> [out_hi, in_lo(128), in_hi, out_lo(128)] bf16."""
    no, ni = Wout_in.shape
    a = Wout_in.reshape(no // 128, 128, ni // 128, 128)
    return _bf(a.transpose(0, 3, 2, 1))


def _repack_cm(Wdf):
    """Wo_cm [D, DFF] -> [d_hi, f_lo(128), f_hi, d_lo(128)] bf16."""
    a = Wdf.reshape(DC, 128, FC, 128)
    return _bf(a.transpose(0, 3, 2, 1))


def _repack_head(Wvd):
    """W [VS, D] -> [NV, d_lo(128), DC, VC] bf16."""
    a = Wvd.reshape(NV, VC, DC, 128)
    return _bf(a.transpose(0, 3, 2, 1))


def kernel(idx, targets, embed, ln_in_w, ln_in_b, Wr, Wk, Wv, Wo_tm, decay,
           gn_w, gn_b, ln1_w, ln1_b, ln2_w, ln2_b, W1, W2, Wo_cm,
           ln_out_w, ln_out_b, exit_ln_w, exit_ln_b, exit_head,
           gate_w1, gate_b1, gate_w2, gate_b2):
    idx = np.asarray(idx)
    targets = np.asarray(targets)
    embed = _f32(embed)
    nc = _get_nc()

    # ---- shared (rank-independent) prep ----
    lnp = np.zeros((128, DC, NLN), np.float32)

    def setln(name, vec):
        lnp[:, :, _LN_COLS[name]] = _featmaj(_f32(vec))

    setln("in_w", ln_in_w); setln("in_b", ln_in_b)
    for i in range(L):
        setln(f"ln1_w_{i}", ln1_w[i]); setln(f"ln1_b_{i}", ln1_b[i])
        setln(f"ln2_w_{i}", ln2_w[i]); setln(f"ln2_b_{i}", ln2_b[i])
    setln("out_w", ln_out_w); setln("out_b", ln_out_b)
    for j in range(2):
        setln(f"ex_w_{j}", exit_ln_w[j]); setln(f"ex_b_{j}", exit_ln_b[j])

    decay = _f32(decay)
    decayT = np.ascontiguousarray(
        np.stack([_featmaj(decay[i]) for i in range(L)], axis=-1))

    gnaux = np.zeros((128, DC, 2 * L), np.float32)
    Wo_tm = _f32(Wo_tm); gn_w = _f32(gn_w); gn_b = _f32(gn_b)
    for i in range(L):
        gnaux[:, :, 2 * i] = _featmaj(Wo_tm[i] @ gn_w[i])
        gnaux[:, :, 2 * i + 1] = _featmaj(Wo_tm[i] @ gn_b[i])

    shared = {"lnp": lnp, "decayT": decayT, "gnaux": gnaux}
    for i in range(L):
        shared[f"wr{i}"] = _repack(_f32(Wr[i]))
        shared[f"wk{i}"] = _repack(_f32(Wk[i]))
        shared[f"wv{i}"] = _repack(_f32(Wv[i]))
        shared[f"wo{i}"] = _repack(Wo_tm[i] * gn_w[i][None, :])
        shared[f"wf1{i}"] = _repack(_f32(W1[i]))
        shared[f"wf2{i}"] = _repack(_f32(W2[i]))
        shared[f"wcm{i}"] = _repack_cm(_f32(Wo_cm[i]))

    x_emb = embed[np.asarray(idx, dtype=np.int64)]        # [B, T, D]
    exit_head = _f32(exit_head)

    in_maps = []
    for r in range(R):
        m = dict(shared)
        sl = x_emb[:, r * TL:(r + 1) * TL, :]              # [B, TL, D]
        m["xembT"] = np.ascontiguousarray(
            sl.transpose(2, 0, 1).reshape(D, NT).astype(np.float32))
        m["iota"] = (np.arange(TL, dtype=np.float32) + r * TL).reshape(1, TL)
        m["rmask"] = (np.arange(R) < r).astype(np.float32).reshape(1, R)
        vs = slice(r * VS, (r + 1) * VS)
        m["headT0"] = _repack_head(exit_head[0, vs, :])
        m["headT1"] = _repack_head(exit_head[1, vs, :])
        m["headT2"] = _repack_head(embed[vs, :])
        in_maps.append(m)

    _CACHE["in_maps"] = in_maps
    res = run_bass_kernel_spmd(nc, in_maps, core_ids=list(range(R)))

    # ---- unshard ----
    def assemble(name):
        shards = []
        for r in range(R):
            a = res.results[r][name]                       # [2048, VS]
            a = a.reshape(R, B, TL, VS).transpose(1, 0, 2, 3).reshape(B, T, VS)
            shards.append(a)
        return np.concatenate(shards, axis=-1)             # [B, T, V]

    final_logits = assemble("logits")
    e_logits = [assemble("exit0"), assemble("exit1")]

    hm = np.zeros((2, B, D), np.float64)
    for r in range(R):
        a = res.results[r]["hemean"]                        # [128, 4, DC]
        for j in range(2):
            for b in range(B):
                hm[j, b] += a[:, j * 2 + b, :].T.reshape(D).astype(np.float64)
    hm /= T

    # ---- loss in f64 ----
    tgt = np.asarray(targets, dtype=np.int64)

    def ce_and_stats(lg):
        lg = lg.astype(np.float64)
        mx = lg.max(-1, keepdims=True)
        ex = np.exp(lg - mx)
        Z = ex.sum(-1, keepdims=True)
        lse = (mx + np.log(Z))[..., 0]                     # [B,T]
        tl_ = np.take_along_axis(lg, tgt[..., None], -1)[..., 0]
        ce = float((lse - tl_).mean())
        pred = lg.argmax(-1)
        p = ex / Z
        ent = lse - (p * lg).sum(-1)                       # [B,T]
        return ce, pred, ent

    ce_f, pred_f, _ = ce_and_stats(final_logits)
    loss = WEIGHTS[-1] * ce_f
    max_ent = np.log(V)
    gate_w1 = _f32(gate_w1); gate_b1 = _f32(gate_b1)
    gate_w2 = _f32(gate_w2); gate_b2 = _f32(gate_b2)
    for j in range(2):
        ce_j, pred_j, ent_j = ce_and_stats(e_logits[j])
        loss += WEIGHTS[j] * ce_j
        agree = (pred_j == pred_f).astype(np.float64).mean(-1, keepdims=True)
        gact = np.maximum(hm[j] @ gate_w1[j].T.astype(np.float64)
                          + gate_b1[j].astype(np.float64), 0.0)
        conf = 1.0 / (1.0 + np.exp(-(gact @ gate_w2[j].T.astype(np.float64)
                                     + gate_b2[j].astype(np.float64))))
        c = np.clip(conf, 1e-7, 1.0 - 1e-7)
        loss += 0.5 * float(-(agree * np.log(c)
                              + (1.0 - agree) * np.log(1.0 - c)).mean())
        overconf = (1.0 - ent_j / max_ent) * (pred_j != pred_f)
        loss += 0.1 * float(overconf.mean())

    return final_logits.astype(np.float32), np.float32(loss)
